# revision 34
# baseline (speedup 1.0000x reference)
"""Distributed Bass/Trainium2 kernel for nn_AreaGNN: 3x SAGEConv(mean) +
global BatchNorm + ReLU, per-graph mean/max pooling, 3-layer MLP head.
SPMD across 8 NeuronCores; takes FULL inputs, returns FULL output [G].

Node layout: batch is sorted, so graphs are contiguous; core c owns graphs
8c..8c+7, each in a fixed SLOT-column window of the feature-major zT (pads
zeroed by a per-core fp8 column mask after each ReLU).  Pooling is then just
uniform per-slot column reduces + one tiny route matmul - no transposes, no
slot gather.  Edges are owned by the dst graph's core, sorted by (src-half,
dst-tile), padded into 128-edge blocks with a global SPMD-uniform schedule.
Layer-0 messages are pre-gathered host-side and streamed; layer-1/2 messages
come from split (A/B) f16 AllGather node tables via hardware dma_gather over
4 SWDGE queues.  The A-half AllGather is issued mid-transpose-loop and the
B-half trigger is deferred into the next layer's chunk loop so gathers for
A-chunks overlap the B AllGather wire time.  Segment sums run on the
TensorEngine via per-block one-hot S matrices (fp8) accumulated in f32 PSUM;
the 1/indeg mean weighting is folded into the per-tile transpose as a
bf16 diagonal matmul.
"""
import numpy as np

N = 50000
E = 800000
D = 128
HID = 128
G = 64
G_FEAT = 32
EPS = 1e-5
NCORES = 8
GPC = G // NCORES           # 8 graphs per core
BLK = 128                   # edges per S block
CBLK = 32                   # blocks per gather chunk (4096 edges)
CH = BLK * CBLK
CPW = CH // 16
NQ = 4                      # SWDGE queues used for gathers
TRIG_B_AFTER = 6            # chunks of next layer emitted before AG_B trigger


# ---------------- host-side preprocessing -----------------------------------

def _wrap_idx(idx, ch):
    """[L] -> [L/ch, 128, ch/16] int16: element m of a chunk at (m%16, m//16),
    replicated across the eight 16-partition groups."""
    L = idx.shape[0]
    out = np.empty((L // ch, 128, ch // 16), dtype=np.int16)
    w = idx.reshape(L // ch, ch // 16, 16).transpose(0, 2, 1)
    for g in range(8):
        out[:, g * 16:(g + 1) * 16, :] = w
    return out


def _preprocess(x, edge_index, batch):
    src = np.asarray(edge_index[0], dtype=np.int64)
    dst = np.asarray(edge_index[1], dtype=np.int64)
    batch = np.asarray(batch, dtype=np.int64)

    # ---- node layout: graph g -> core g//GPC, slot g%GPC ----
    cnt_g = np.bincount(batch, minlength=G)
    assert cnt_g.min() >= 1, "empty graph unsupported"
    gstart = np.concatenate([[0], np.cumsum(cnt_g)])
    maxsz = int(cnt_g.max())
    SLOT = max(896, -(-(maxsz + 2) // 128) * 128)   # last col of every slot pad
    NCOL = GPC * SLOT
    NTILES = NCOL // 128
    NAH = NCOL // 2                                 # cols per table half
    TROWS = NAH                                     # rows per core per table
    assert NCORES * TROWS < 32768                   # int16 gather indices

    g_of = batch
    core_of_node = g_of // GPC
    newcol = (g_of % GPC) * SLOT + (np.arange(N) - gstart[g_of])

    indeg = np.bincount(dst, minlength=N)
    invdeg_all = (1.0 / np.maximum(indeg, 1.0)).astype(np.float32)

    dstc = core_of_node[dst]
    dcol = newcol[dst]
    tile_of = dcol // 128
    scol = newcol[src]
    score = core_of_node[src]
    half_of = (scol >= NAH).astype(np.int64)
    src_pos = score * TROWS + (scol - half_of * NAH)  # position in half table

    # per (core, half, tile) edge lists
    counts = np.zeros((NCORES, 2, NTILES), dtype=np.int64)
    buckets = {}
    for c in range(NCORES):
        mc = dstc == c
        for h in range(2):
            mh = mc & (half_of == h)
            for t in range(NTILES):
                m = mh & (tile_of == t)
                buckets[(c, h, t)] = (src_pos[m], dcol[m] % 128, src[m])
                counts[c, h, t] = int(m.sum())

    # global block schedule: NBLK[h, t] = max over cores; half A gets >= 1
    # block per tile so the pass-A psum->agg copy always initializes agg cols
    nblk = np.ceil(counts.max(axis=0) / BLK).astype(np.int64)  # [2, NTILES]
    nblk[0] = np.maximum(nblk[0], 1)
    extra = [0, 0]
    for h in range(2):
        tot = int(nblk[h].sum())
        extra[h] = (-tot) % CBLK
    sched = []   # list of (h, t) per block, in execution order
    for h in range(2):
        for t in range(NTILES):
            sched += [(h, t)] * int(nblk[h, t])
        sched += [(h, NTILES - 1)] * extra[h]
    nblk_tot = len(sched)
    nchunks = nblk_tot // CBLK
    assert nchunks * CBLK == nblk_tot
    chunk_half = [sched[k * CBLK][0] for k in range(nchunks)]
    for k in range(nchunks):
        assert all(sched[k * CBLK + j][0] == chunk_half[k] for j in range(CBLK))

    # Real (max-over-cores) block prefix of each (h,t) run: only these blocks
    # are matmul'd, and gather pieces cover exactly their units, so no stale
    # SBUF is ever read.  Pure-padding blocks are skipped entirely.
    run_start = {}
    b0 = 0
    for h in range(2):
        for t in range(NTILES):
            nb = int(nblk[h, t]) + (extra[h] if t == NTILES - 1 else 0)
            run_start[(h, t)] = (b0, nb)
            b0 += nb
    nbr = {}   # real blocks per run (>=1 for half A: initializes agg cols)
    for (h, t), (rb, nb) in run_start.items():
        n = -(-int(counts[:, h, t].max()) // BLK)
        if h == 0:
            n = max(n, 1)
        nbr[(h, t)] = min(n, nb)
    use_block = np.zeros(nblk_tot, dtype=bool)
    blk_start = np.zeros(nblk_tot, dtype=bool)
    blk_stop = np.zeros(nblk_tot, dtype=bool)
    for (h, t), (rb, nb) in run_start.items():
        n = nbr[(h, t)]
        if n > 0:
            use_block[rb:rb + n] = True
            blk_start[rb] = True
            blk_stop[rb + n - 1] = True
    # pieces trimmed to per-run real extents (128-aligned, <=1024 idx each)
    gather_pieces = []
    for k in range(nchunks):
        c0, c1 = k * CBLK * BLK, (k + 1) * CBLK * BLK
        pieces = []
        for (h, t), (rb, nb) in run_start.items():
            if h != chunk_half[k]:
                continue
            s0, s1 = rb * BLK, rb * BLK + nbr[(h, t)] * BLK
            a, b = max(s0, c0), min(s1, c1)
            for s in range(a, b, 1024):
                pieces.append((s - c0, min(1024, b - s)))
        gather_pieces.append(sorted(pieces))
    # chunk after which tile t's aggregation is complete
    last_blk = {}
    for (h, t), (rb, nb) in run_start.items():
        n = nbr[(h, t)]
        if n > 0:
            last_blk[t] = max(last_blk.get(t, -1), rb + n - 1)
    tile_ready = [last_blk[t] // CBLK for t in range(NTILES)]

    # per-core gather idx + per-slot dst columns following the schedule.
    # S blocks are pure one-hot rows, generated ON-CHIP per block via a DVE
    # is_equal against dcol (slot -> dst column, -1 for pad slots); the
    # 1/indeg mean weight is applied later by a diagonal-matmul fused into
    # the per-tile transpose.
    import ml_dtypes
    gidx_cores, dcol_cores, giabs_cores, diag_cores = [], [], [], []
    mask_cores, R2d_cores, S_cores = [], [], []
    inv_cnt = (1.0 / cnt_g.astype(np.float64)).astype(np.float32)
    for c in range(NCORES):
        gi = np.zeros(nblk_tot * BLK, dtype=np.int64)
        ga = np.zeros(nblk_tot * BLK, dtype=np.int64)   # abs src (msgs0)
        dc = np.full(nblk_tot * BLK, -1.0, dtype=np.float32)
        b0 = 0
        for h in range(2):
            for t in range(NTILES):
                nb = int(nblk[h, t]) + (extra[h] if t == NTILES - 1 else 0)
                g, d, a = buckets[(c, h, t)]
                n = len(g)
                gi[b0 * BLK: b0 * BLK + n] = g
                ga[b0 * BLK: b0 * BLK + n] = a
                dc[b0 * BLK: b0 * BLK + n] = d
                b0 += nb
        assert b0 == nblk_tot
        giabs_cores.append(ga)
        gidx_cores.append(_wrap_idx(gi.astype(np.int16), CH))
        # dcol layout [slot-in-block (partition), block]
        dcol_cores.append(np.ascontiguousarray(
            dc.reshape(nblk_tot, BLK).T))
        # dense one-hot S, streamed only by layer 0 (DMA has headroom there;
        # gather-bound layers 1/2 generate S on-chip instead)
        S = np.zeros((nblk_tot * BLK, 128), dtype=np.float32)
        rows = np.arange(nblk_tot * BLK)
        real = dc >= 0
        S[rows[real], dc[real].astype(np.int64)] = 1.0
        S_cores.append(np.ascontiguousarray(
            S.reshape(nblk_tot, BLK, 128).transpose(1, 0, 2)
            .reshape(BLK, nblk_tot * 128)).astype(ml_dtypes.float8_e4m3))

        iv = np.ones(NCOL, np.float32)
        msk = np.zeros(NCOL, np.float32)
        for r in range(GPC):
            g = c * GPC + r
            sz = int(cnt_g[g])
            iv[r * SLOT:r * SLOT + sz] = \
                invdeg_all[gstart[g]:gstart[g] + sz]
            msk[r * SLOT:r * SLOT + sz] = 1.0
        dg = np.zeros((128, NCOL), dtype=np.float32)
        for t in range(NTILES):
            dg[np.arange(128), t * 128 + np.arange(128)] = \
                iv[t * 128:(t + 1) * 128]
        diag_cores.append(dg.astype(ml_dtypes.bfloat16))
        mask_cores.append(
            np.broadcast_to(msk.astype(ml_dtypes.float8_e4m3),
                            (128, NCOL)).copy())

        R2d = np.zeros((2 * GPC, 2 * G), dtype=np.float32)
        for r in range(GPC):
            g = c * GPC + r
            R2d[r, g] = inv_cnt[g]
            R2d[GPC + r, G + g] = 1.0
        R2d_cores.append(R2d)

    return dict(SLOT=SLOT, NCOL=NCOL, NTILES=NTILES, NAH=NAH, TROWS=TROWS,
                cnt_g=cnt_g, gstart=gstart,
                nblk=nblk, extra=extra, sched=sched, nblk_tot=nblk_tot,
                nchunks=nchunks, chunk_half=chunk_half,
                use_block=use_block, blk_start=blk_start, blk_stop=blk_stop,
                gather_pieces=gather_pieces, tile_ready=tile_ready,
                gidx=gidx_cores, giabs=giabs_cores, dcol=dcol_cores,
                S=S_cores,
                diag=diag_cores, mask=mask_cores, R2d=R2d_cores)


# ---------------- device kernel builder --------------------------------------

def _build(nc, pre):
    import concourse.mybir as mybir
    import concourse.tile as tile

    f32 = mybir.dt.float32
    f16 = mybir.dt.float16
    bf16 = mybir.dt.bfloat16
    f8 = mybir.dt.float8e4
    i16 = mybir.dt.int16
    NCH = pre['nchunks']
    NBLK_TOT = pre['nblk_tot']
    NCOL, NTILES, NAH, SLOT = \
        pre['NCOL'], pre['NTILES'], pre['NAH'], pre['SLOT']
    TROWS = pre['TROWS']
    HROWS = NCORES * TROWS      # rows per (A or B) gather table
    HT = NTILES // 2            # tiles per half
    SGRP = SLOT // 128          # tiles per bounce-ship DMA (= tiles/slot)
    NGRP = HT // SGRP           # = GPC // 2
    assert NGRP * SGRP == HT
    sched = pre['sched']

    use_block = pre['use_block']
    is_start = pre['blk_start']
    is_stop = pre['blk_stop']

    # ---- I/O ----
    msgs0_d = nc.dram_tensor("msgs0", [NCH, 128, CBLK * D], f16,
                             kind="ExternalInput")
    xownT = nc.dram_tensor("xownT", [D, NCOL], f32, kind="ExternalInput")
    gidx_d = nc.dram_tensor("gidx", [NCH, 128, CPW], i16, kind="ExternalInput")
    S_d = nc.dram_tensor("S", [BLK, NBLK_TOT * 128], f8, kind="ExternalInput")
    dcol_d = nc.dram_tensor("dcol", [BLK, NBLK_TOT], f32, kind="ExternalInput")
    colidx_d = nc.dram_tensor("colidx", [128, 128], f32, kind="ExternalInput")
    diag_d = nc.dram_tensor("diag", [128, NCOL], bf16, kind="ExternalInput")
    mask_d = nc.dram_tensor("mask", [128, NCOL], f8, kind="ExternalInput")
    R2d_d = nc.dram_tensor("R2d", [2 * GPC, 2 * G], f32, kind="ExternalInput")
    gfT_d = nc.dram_tensor("gfT", [G_FEAT, G], f32, kind="ExternalInput")
    ident_d = nc.dram_tensor("ident", [128, 128], f32, kind="ExternalInput")
    Wl_d = [nc.dram_tensor(f"Wl{i}", [D, HID], f32, kind="ExternalInput")
            for i in range(3)]
    Wr_d = [nc.dram_tensor(f"Wr{i}", [D, HID], f32, kind="ExternalInput")
            for i in range(3)]
    gb_d = [nc.dram_tensor(f"gb{i}", [HID, 2], f32, kind="ExternalInput")
            for i in range(3)]
    W1_d = nc.dram_tensor("W1", [2 * HID + G_FEAT, HID], f32, kind="ExternalInput")
    W2_d = nc.dram_tensor("W2", [HID, HID // 2], f32, kind="ExternalInput")
    W3_d = nc.dram_tensor("W3", [HID // 2, 1], f32, kind="ExternalInput")
    bT_d = nc.dram_tensor("bT", [HID, 3], f32, kind="ExternalInput")

    out_d = nc.dram_tensor("out", [G], f32, kind="ExternalOutput")

    rg = [list(range(NCORES))]

    with tile.TileContext(nc) as tc:
        with (
            tc.tile_pool(name="sb", bufs=3) as sb,
            tc.tile_pool(name="big", bufs=2) as bigp,       # zT (f32 NCOL)
            tc.tile_pool(name="agg", bufs=1) as aggp,       # agg_sb bf16
            tc.tile_pool(name="big1", bufs=1) as big1,      # allp
            tc.tile_pool(name="msg", bufs=3) as msgp,       # msgs + S per chunk
            tc.tile_pool(name="idx", bufs=6) as idxp,
            tc.tile_pool(name="stg", bufs=2) as stgp,       # bounce staging
            tc.tile_pool(name="cst", bufs=1) as cst,
            tc.tile_pool(name="ps", bufs=2, space="PSUM") as ps,
            tc.tile_pool(name="pst", bufs=2, space="PSUM") as pst,
            tc.tile_pool(name="psa", bufs=2, space="PSUM") as psa,
            tc.tile_pool(name="psm", bufs=1, space="PSUM") as psm,
            tc.tile_pool(name="dram", bufs=1, space="DRAM") as dram,
        ):
            # ---- DRAM scratch ----
            hbA = [dram.tile([TROWS, D], f16, tag=f"hbA{i}", name=f"hbA{i}")
                   for i in range(2)]
            hbB = [dram.tile([TROWS, D], f16, tag=f"hbB{i}", name=f"hbB{i}")
                   for i in range(2)]
            tblA = [dram.tile([HROWS, D], f16, tag=f"tblA{i}",
                              name=f"tblA{i}", addr_space="Shared")
                    for i in range(2)]
            tblB = [dram.tile([HROWS, D], f16, tag=f"tblB{i}",
                              name=f"tblB{i}", addr_space="Shared")
                    for i in range(2)]
            stats_in = [dram.tile([D, 2], f32, tag=f"stats_in{i}",
                                  name=f"stats_in{i}") for i in range(3)]
            stats_out = [dram.tile([NCORES * D, 2], f32, tag=f"stats_out{i}",
                                   name=f"stats_out{i}", addr_space="Shared")
                         for i in range(3)]
            pool_in = dram.tile([D, 2 * G], f32, tag="pool_in")
            pool_out = dram.tile([NCORES * D, 2 * G], f32, tag="pool_out",
                                 addr_space="Shared")

            def load_const(src_ap, rows, cols, name, dt=f32):
                t = cst.tile([rows, cols], dt, tag=name)
                nc.sync.dma_start(out=t[:, :], in_=src_ap)
                return t

            ident_sb = load_const(ident_d[:, :], 128, 128, "ident")
            diag_sb = load_const(diag_d[:, :], 128, NCOL, "diag", bf16)
            mask_sb = load_const(mask_d[:, :], 128, NCOL, "mask", f8)
            dcol_sb = load_const(dcol_d[:, :], BLK, NBLK_TOT, "dcol")
            colidx_sb = load_const(colidx_d[:, :], 128, 128, "colidx")
            xT_sb = bigp.tile([128, NCOL], f32, tag="zT")
            nc.sync.dma_start(out=xT_sb[:, :], in_=xownT[:, :])

            hT_prev = xT_sb

            # deferred AG_B trigger (emitted inside the NEXT layer's chunk
            # loop so A-chunk gathers overlap the B AllGather wire time)
            pending_agb = [None]

            def emit_agb():
                if pending_agb[0] is not None:
                    hb, tb = pending_agb[0]
                    nc.gpsimd.collective_compute(
                        "AllGather", mybir.AluOpType.bypass,
                        replica_groups=rg, ins=[hb.opt()], outs=[tb.opt()])
                    pending_agb[0] = None

            for li in range(3):
                Wl_sb = load_const(Wl_d[li][:, :], D, HID, f"Wl{li}")
                Wr_sb = load_const(Wr_d[li][:, :], D, HID, f"Wr{li}")
                gb_sb = load_const(gb_d[li][:, :], HID, 2, f"gb{li}")

                agg_sb = aggp.tile([128, NCOL], bf16, tag="agg")
                zT = bigp.tile([128, NCOL], f32, tag="zT")
                zsum = sb.tile([128, NTILES], f32, tag="zsum")
                zsq = sb.tile([128, NTILES], f32, tag="zsq")
                sq_scr = sb.tile([128, D], f32, tag="sqscr")

                def dense_tile(t, zT=zT, zsum=zsum, zsq=zsq, sq_scr=sq_scr,
                               agg_sb=agg_sb, Wl_sb=Wl_sb, Wr_sb=Wr_sb,
                               hT_prev=hT_prev):
                    aT_ps = pst.tile([128, D], f32, tag="tp")
                    # transpose fused with the 1/indeg column scale:
                    # out[f, j] = sum_k agg[k, f] * diag[k, j] = agg[j, f]/deg_j
                    nc.tensor.matmul(aT_ps[:, :],
                                     agg_sb[:, t * 128:(t + 1) * 128],
                                     diag_sb[:, t * 128:(t + 1) * 128],
                                     start=True, stop=True)
                    aT_sb = sb.tile([128, D], f32, tag="aTs")
                    nc.vector.tensor_copy(aT_sb[:, :], aT_ps[:, :])
                    z_ps = ps.tile([128, D], f32, tag="z")
                    nc.tensor.matmul(z_ps[:, :], Wl_sb[:, :], aT_sb[:, :],
                                     start=True, stop=False)
                    nc.tensor.matmul(z_ps[:, :], Wr_sb[:, :],
                                     hT_prev[:, t * 128:(t + 1) * 128],
                                     start=False, stop=True)
                    nc.scalar.activation(zT[:, t * 128:(t + 1) * 128],
                                         z_ps[:, :],
                                         mybir.ActivationFunctionType.Copy,
                                         accum_out=zsum[:, t:t + 1])
                    nc.scalar.activation(sq_scr[:, :],
                                         zT[:, t * 128:(t + 1) * 128],
                                         mybir.ActivationFunctionType.Square,
                                         accum_out=zsq[:, t:t + 1])

                # ---- gather + S-matmul aggregation, dense tiles interleaved
                # into the chunk loop as their aggregation completes (engine
                # queues execute in emission order - this is what overlaps
                # the dense phase with the gather pipeline) ----
                acc_ps = None
                gq = [0]
                tdone = 0
                for k in range(NCH):
                    h = pre['chunk_half'][k]
                    if li > 0 and h == 1:
                        emit_agb()   # half-B gathers need the B AllGather
                    msgs = msgp.tile([128, CBLK, D], f16, tag="msgs")
                    if li == 0:
                        nc.sync.dma_start(
                            out=msgs[:, :, :],
                            in_=msgs0_d[k, :, :].rearrange(
                                "p (j d) -> p j d", d=D))
                    else:
                        src_tab = (tblA if h == 0 else tblB)[(li - 1) % 2]
                        gi_t = idxp.tile([128, CPW], i16, tag="gi")
                        nc.sync.dma_start(out=gi_t[:], in_=gidx_d[k, :, :])
                        # runtime faults on dma_gather num_idxs > 1024:
                        # <=1024-index sub-gathers trimmed to per-run real
                        # extents; round-robin the 4 SWDGE queues
                        for off, n in pre['gather_pieces'][k]:
                            nc.gpsimd.dma_gather(
                                msgs[:, off // 128:off // 128
                                     + (n + 127) // 128, :],
                                src_tab[0:HROWS, :],
                                gi_t[:, off // 16:off // 16 + n // 16],
                                n, n, D, queue_num=gq[0] % NQ,
                                single_packet=False)
                            gq[0] += 1
                    # S blocks are one-hot.  Layer 0 streams them (its DMA
                    # has headroom); gather-bound layers 1/2 generate them
                    # on-chip (DVE is_equal of the column-iota against dcol)
                    S_t = msgp.tile([128, CBLK, D], f8, tag="Ssb")
                    if li == 0:
                        nc.scalar.dma_start(
                            out=S_t[:, :, :],
                            in_=S_d[:, k * CBLK * 128:(k + 1) * CBLK * 128]
                            .rearrange("p (j d) -> p j d", d=128))
                    else:
                        for j in range(CBLK):
                            b = k * CBLK + j
                            if not use_block[b]:
                                continue
                            nc.vector.scalar_tensor_tensor(
                                S_t[:, j, :], colidx_sb[:, :],
                                dcol_sb[:, b:b + 1], colidx_sb[:, :],
                                mybir.AluOpType.is_equal,
                                mybir.AluOpType.bypass)
                    for j in range(CBLK):
                        b = k * CBLK + j
                        if not use_block[b]:
                            continue
                        h_b, t_b = sched[b]
                        if is_start[b]:
                            acc_ps = psa.tile([128, D], f32, tag="accp")
                        nc.tensor.matmul(acc_ps[:, :], S_t[:, j, :],
                                         msgs[:, j, :],
                                         start=is_start[b], stop=is_stop[b])
                        if is_stop[b]:
                            if h_b == 0:
                                nc.scalar.copy(
                                    agg_sb[:, t_b * 128:(t_b + 1) * 128],
                                    acc_ps[:, :])
                            else:
                                nc.vector.tensor_add(
                                    agg_sb[:, t_b * 128:(t_b + 1) * 128],
                                    agg_sb[:, t_b * 128:(t_b + 1) * 128],
                                    acc_ps[:, :])
                    if li > 0 and k == TRIG_B_AFTER:
                        emit_agb()
                    while tdone < NTILES and pre['tile_ready'][tdone] <= k:
                        dense_tile(tdone)
                        tdone += 1
                emit_agb()
                while tdone < NTILES:
                    dense_tile(tdone)
                    tdone += 1

                stat_sb = sb.tile([128, 2], f32, tag="stat")
                nc.vector.tensor_reduce(stat_sb[:, 0:1], zsum[:, :],
                                        mybir.AxisListType.X,
                                        mybir.AluOpType.add)
                nc.vector.tensor_reduce(stat_sb[:, 1:2], zsq[:, :],
                                        mybir.AxisListType.X,
                                        mybir.AluOpType.add)
                nc.sync.dma_start(out=stats_in[li][:, :], in_=stat_sb[:, :])
                nc.gpsimd.collective_compute(
                    "AllGather", mybir.AluOpType.bypass, replica_groups=rg,
                    ins=[stats_in[li].opt()], outs=[stats_out[li].opt()])
                allst = sb.tile([128, NCORES, 2], f32, tag="allst")
                nc.sync.dma_start(
                    out=allst[:, :, :],
                    in_=stats_out[li][:, :].rearrange("(c p) j -> p c j",
                                                      c=NCORES))
                tot = sb.tile([128, 2], f32, tag="tot")
                nc.vector.tensor_add(tot[:, :], allst[:, 0, :], allst[:, 1, :])
                for c in range(2, NCORES):
                    nc.vector.tensor_add(tot[:, :], tot[:, :], allst[:, c, :])
                mu = sb.tile([128, 6], f32, tag="mu")
                nc.scalar.mul(mu[:, 0:1], tot[:, 0:1], 1.0 / N)
                nc.scalar.mul(mu[:, 1:2], tot[:, 1:2], 1.0 / N)
                nc.vector.tensor_mul(mu[:, 2:3], mu[:, 0:1], mu[:, 0:1])
                nc.vector.tensor_sub(mu[:, 3:4], mu[:, 1:2], mu[:, 2:3])
                nc.vector.tensor_scalar_add(mu[:, 3:4], mu[:, 3:4], EPS)
                nc.vector.reciprocal(mu[:, 4:5], mu[:, 3:4])
                nc.scalar.sqrt(mu[:, 4:5], mu[:, 4:5])
                nc.vector.tensor_mul(mu[:, 4:5], mu[:, 4:5], gb_sb[:, 0:1])
                nc.vector.tensor_mul(mu[:, 5:6], mu[:, 0:1], mu[:, 4:5])
                nc.vector.tensor_sub(mu[:, 5:6], gb_sb[:, 1:2], mu[:, 5:6])
                # h = relu(z*s + shift), pads zeroed via the fp8 column mask
                # (pads must stay 0 for BN stats + pooling), interleaved
                # per-tile with the transpose-ship / pooling consumers
                def apply_tile(t, zT=zT, mu=mu):
                    sl = slice(t * 128, (t + 1) * 128)
                    nc.scalar.activation(zT[:, sl], zT[:, sl],
                                         mybir.ActivationFunctionType.Relu,
                                         bias=mu[:, 5:6], scale=mu[:, 4:5])
                    nc.vector.tensor_mul(zT[:, sl], zT[:, sl], mask_sb[:, sl])
                hT_prev = zT

                if li < 2:
                    # ---- transpose back; ship f16 halves to split AG ----
                    for half, hb, tb in ((0, hbA[li % 2], tblA[li % 2]),
                                         (1, hbB[li % 2], tblB[li % 2])):
                        for grp in range(NGRP):
                            stg = stgp.tile([128, SGRP, 128], f16, tag="stg")
                            for j in range(SGRP):
                                t = half * HT + grp * SGRP + j
                                apply_tile(t)
                                hT_ps = pst.tile([128, D], f32, tag="tp")
                                nc.tensor.transpose(
                                    hT_ps[:, :],
                                    zT[:, t * 128:(t + 1) * 128],
                                    ident_sb[:, :])
                                nc.vector.tensor_copy(stg[:, j, :],
                                                      hT_ps[:, :])
                            r0 = grp * SGRP * 128
                            nc.sync.dma_start(
                                out=hb[r0:r0 + SGRP * 128, :].rearrange(
                                    "(j p) d -> p j d", p=128),
                                in_=stg[:, :, :])
                        if half == 0:
                            nc.gpsimd.collective_compute(
                                "AllGather", mybir.AluOpType.bypass,
                                replica_groups=rg,
                                ins=[hb.opt()], outs=[tb.opt()])
                        else:
                            pending_agb[0] = (hb, tb)
                else:
                    # ---- pooling: uniform per-slot column reduces ----
                    loc = sb.tile([128, 2 * GPC], f32, tag="loc")
                    for r in range(GPC):
                        for j in range(SGRP):
                            apply_tile(r * SGRP + j)
                        sl = slice(r * SLOT, (r + 1) * SLOT)
                        nc.vector.tensor_reduce(loc[:, r:r + 1], zT[:, sl],
                                                mybir.AxisListType.X,
                                                mybir.AluOpType.add)
                        nc.vector.tensor_reduce(loc[:, GPC + r:GPC + r + 1],
                                                zT[:, sl],
                                                mybir.AxisListType.X,
                                                mybir.AluOpType.max)
                    locT_ps = pst.tile([2 * GPC, 128], f32, tag="tp",
                                       name="locT")
                    nc.tensor.transpose(locT_ps[:, :], loc[:, :],
                                        ident_sb[:, :])
                    locT_sb = sb.tile([2 * GPC, 128], f32, tag="locTs")
                    nc.vector.tensor_copy(locT_sb[:, :], locT_ps[:, :])
                    R2d_sb = load_const(R2d_d[:, :], 2 * GPC, 2 * G, "R2d")
                    pool_ps = psm.tile([128, 2 * G], f32, tag="tail")
                    nc.tensor.matmul(pool_ps[:, :], locT_sb[:, :],
                                     R2d_sb[:, :], start=True, stop=True)

            # ---- pool partial exchange ----
            pool_sb = sb.tile([128, 2 * G], f32, tag="poolp")
            nc.vector.tensor_copy(pool_sb[:, :], pool_ps[:, :])
            nc.sync.dma_start(out=pool_in[:, :], in_=pool_sb[:, :])
            nc.gpsimd.collective_compute(
                "AllGather", mybir.AluOpType.bypass, replica_groups=rg,
                ins=[pool_in.opt()], outs=[pool_out.opt()])
            allp = big1.tile([128, NCORES, 2 * G], f32, tag="allp")
            nc.sync.dma_start(
                out=allp[:, :, :],
                in_=pool_out[:, :].rearrange("(c p) j -> p c j", c=NCORES))
            meanTot = sb.tile([128, G], f32, tag="meanTot")
            maxTot = sb.tile([128, G], f32, tag="maxTot")
            nc.vector.tensor_add(meanTot[:, :], allp[:, 0, 0:G],
                                 allp[:, 1, 0:G])
            nc.vector.tensor_max(maxTot[:, :], allp[:, 0, G:2 * G],
                                 allp[:, 1, G:2 * G])
            for c in range(2, NCORES):
                nc.vector.tensor_add(meanTot[:, :], meanTot[:, :],
                                     allp[:, c, 0:G])
                nc.vector.tensor_max(maxTot[:, :], maxTot[:, :],
                                     allp[:, c, G:2 * G])

            # ---- head (feature-major) ----
            W1a_sb = load_const(W1_d[0:HID, :], HID, HID, "W1a")
            W1b_sb = load_const(W1_d[HID:2 * HID, :], HID, HID, "W1b")
            W1c_sb = load_const(W1_d[2 * HID:2 * HID + G_FEAT, :], G_FEAT,
                                HID, "W1c")
            W2_sb = load_const(W2_d[:, :], HID, HID // 2, "W2")
            W3_sb = load_const(W3_d[:, :], HID // 2, 1, "W3")
            bT_sb = load_const(bT_d[:, :], HID, 3, "bT")
            gfT_sb = load_const(gfT_d[:, :], G_FEAT, G, "gfT")

            m1_ps = psm.tile([HID, G], f32, tag="tail")
            nc.tensor.matmul(m1_ps[:, :], W1a_sb[:, :], meanTot[:, :],
                             start=True, stop=False)
            nc.tensor.matmul(m1_ps[:, :], W1b_sb[:, :], maxTot[:, :],
                             start=False, stop=False)
            nc.tensor.matmul(m1_ps[:, :], W1c_sb[:, :],
                             gfT_sb[:, :], start=False, stop=True)
            m1_sb = sb.tile([HID, G], f32, tag="m1s")
            nc.scalar.activation(m1_sb[:, :], m1_ps[:, :],
                                 mybir.ActivationFunctionType.Relu,
                                 bias=bT_sb[:, 0:1])
            m2_ps = psm.tile([HID // 2, G], f32, tag="tail")
            nc.tensor.matmul(m2_ps[:, :], W2_sb[:, :], m1_sb[:, :],
                             start=True, stop=True)
            m2_sb = sb.tile([HID // 2, G], f32, tag="m2s")
            nc.scalar.activation(m2_sb[:, :], m2_ps[:, :],
                                 mybir.ActivationFunctionType.Relu,
                                 bias=bT_sb[0:HID // 2, 1:2])
            m3_ps = psm.tile([1, G], f32, tag="tail")
            nc.tensor.matmul(m3_ps[:, :], W3_sb[:, :], m2_sb[:, :],
                             start=True, stop=True)
            m3_sb = sb.tile([1, G], f32, tag="m3s")
            nc.scalar.copy(m3_sb[:, :], m3_ps[:, :])
            nc.vector.tensor_scalar_add(m3_sb[:, :], m3_sb[:, :],
                                        bT_sb[0:1, 2:3])
            nc.sync.dma_start(out=out_d[:].rearrange("(o g) -> o g", o=1),
                              in_=m3_sb[:, :])
    return nc


# ---------------- public entry ------------------------------------------------

def build_in_maps(x, edge_index, batch, g_feats, params, pre):
    x = np.asarray(x, dtype=np.float32)
    g_feats = np.asarray(g_feats, dtype=np.float32)
    batch = np.asarray(batch, dtype=np.int64)

    bT = np.zeros((HID, 3), np.float32)
    bT[:, 0] = np.asarray(params['b1'], np.float32)
    bT[:HID // 2, 1] = np.asarray(params['b2'], np.float32)
    bT[0, 2] = np.asarray(params['b3'], np.float32).reshape(-1)[0]

    common = {
        "ident": np.eye(128, dtype=np.float32),
        "colidx": np.broadcast_to(
            np.arange(128, dtype=np.float32), (128, 128)).copy(),
        "gfT": np.ascontiguousarray(g_feats.T),
        "W1": np.asarray(params['W1'], np.float32),
        "W2": np.asarray(params['W2'], np.float32),
        "W3": np.asarray(params['W3'], np.float32),
        "bT": bT,
    }
    for i in range(3):
        common[f"Wl{i}"] = np.asarray(params[f'Wl{i}'], np.float32)
        common[f"Wr{i}"] = np.asarray(params[f'Wr{i}'], np.float32)
        gb = np.zeros((HID, 2), np.float32)
        gb[:, 0] = np.asarray(params[f'gamma{i}'], np.float32)
        gb[:, 1] = np.asarray(params[f'beta{i}'], np.float32)
        common[f"gb{i}"] = gb

    x16 = x.astype(np.float16)
    NCH = pre['nchunks']
    SLOT, NCOL = pre['SLOT'], pre['NCOL']
    cnt_g, gstart = pre['cnt_g'], pre['gstart']
    in_maps = []
    for c in range(NCORES):
        xo = np.zeros((NCOL, D), np.float32)
        for r in range(GPC):
            g = c * GPC + r
            sz = int(cnt_g[g])
            xo[r * SLOT:r * SLOT + sz] = x[gstart[g]:gstart[g] + sz]
        # pre-gather layer-0 messages into the exact chunk SBUF layout:
        # slot s of chunk k -> partition s%128, free block s//128
        gi_abs = pre['giabs'][c]
        msgs0 = x16[gi_abs].reshape(NCH, CBLK, BLK, D).transpose(0, 2, 1, 3)
        msgs0 = np.ascontiguousarray(msgs0.reshape(NCH, 128, CBLK * D))
        m = dict(common)
        m.update({
            "xownT": np.ascontiguousarray(xo.T),
            "msgs0": msgs0,
            "gidx": pre['gidx'][c],
            "S": pre['S'][c],
            "dcol": pre['dcol'][c],
            "diag": pre['diag'][c],
            "mask": pre['mask'][c],
            "R2d": pre['R2d'][c],
        })
        in_maps.append(m)
    return in_maps


def build_nc(pre):
    import os
    import concourse.bacc as bacc
    nc = bacc.Bacc(None, target_bir_lowering=False, debug=False,
                   num_devices=NCORES, num_swdge_queues=4,
                   detect_race_conditions=os.environ.get(
                       "KERNEL_NO_RACE_CHECK") != "1")
    nc = _build(nc, pre)
    nc.compile()
    return nc


def kernel(x, edge_index, batch, g_feats,
           Wl0, bl0, Wr0, gamma0, beta0,
           Wl1, bl1, Wr1, gamma1, beta1,
           Wl2, bl2, Wr2, gamma2, beta2,
           W1, b1, W2, b2, W3, b3):
    # bl{i} cancels inside BatchNorm (constant pre-BN shift), so it is unused.
    from concourse.bass_utils import run_bass_kernel_spmd

    params = dict(Wl0=Wl0, Wr0=Wr0, gamma0=gamma0, beta0=beta0,
                  Wl1=Wl1, Wr1=Wr1, gamma1=gamma1, beta1=beta1,
                  Wl2=Wl2, Wr2=Wr2, gamma2=gamma2, beta2=beta2,
                  W1=W1, b1=b1, W2=W2, b2=b2, W3=W3, b3=b3)
    pre = _preprocess(x, edge_index, batch)
    nc = build_nc(pre)
    in_maps = build_in_maps(x, edge_index, batch, g_feats, params, pre)
    res = run_bass_kernel_spmd(nc, in_maps, list(range(NCORES)))
    return np.asarray(res.results[0]["out"], dtype=np.float32)


# revision 35
# speedup vs baseline: 1.0700x; 1.0700x over previous
"""Distributed Bass/Trainium2 kernel for nn_AreaGNN: 3x SAGEConv(mean) +
global BatchNorm + ReLU, per-graph mean/max pooling, 3-layer MLP head.
SPMD across 8 NeuronCores; takes FULL inputs, returns FULL output [G].

Node layout: batch is sorted, so graphs are contiguous; core c owns graphs
8c..8c+7, each in a fixed SLOT-column window of the feature-major zT (pads
zeroed by a per-core fp8 column mask after each ReLU).  Pooling is then just
uniform per-slot column reduces + one tiny route matmul - no transposes, no
slot gather.  Edges are owned by the dst graph's core, sorted by (src-half,
dst-tile), padded into 128-edge blocks with a global SPMD-uniform schedule.
Layer-0 messages are pre-gathered host-side and streamed; layer-1/2 messages
come from split (A/B) f16 AllGather node tables via hardware dma_gather over
4 SWDGE queues.  The A-half AllGather is issued mid-transpose-loop and the
B-half trigger is deferred into the next layer's chunk loop so gathers for
A-chunks overlap the B AllGather wire time.  Segment sums run on the
TensorEngine via per-block one-hot S matrices (fp8) accumulated in f32 PSUM;
the 1/indeg mean weighting is folded into the per-tile transpose as a
bf16 diagonal matmul.
"""
import numpy as np

N = 50000
E = 800000
D = 128
HID = 128
G = 64
G_FEAT = 32
EPS = 1e-5
NCORES = 8
GPC = G // NCORES           # 8 graphs per core
BLK = 128                   # edges per S block
CBLK = 32                   # blocks per gather chunk (4096 edges)
CH = BLK * CBLK
CPW = CH // 16
NQ = 4                      # SWDGE queues used for gathers
TRIG_B_AFTER = 6            # chunks of next layer emitted before AG_B trigger


# ---------------- host-side preprocessing -----------------------------------

def _wrap_idx(idx, ch):
    """[L] -> [L/ch, 128, ch/16] int16: element m of a chunk at (m%16, m//16),
    replicated across the eight 16-partition groups."""
    L = idx.shape[0]
    out = np.empty((L // ch, 128, ch // 16), dtype=np.int16)
    w = idx.reshape(L // ch, ch // 16, 16).transpose(0, 2, 1)
    for g in range(8):
        out[:, g * 16:(g + 1) * 16, :] = w
    return out


def _preprocess(x, edge_index, batch):
    src = np.asarray(edge_index[0], dtype=np.int64)
    dst = np.asarray(edge_index[1], dtype=np.int64)
    batch = np.asarray(batch, dtype=np.int64)

    # ---- node layout: graph g -> core g//GPC, slot g%GPC ----
    cnt_g = np.bincount(batch, minlength=G)
    assert cnt_g.min() >= 1, "empty graph unsupported"
    gstart = np.concatenate([[0], np.cumsum(cnt_g)])
    maxsz = int(cnt_g.max())
    SLOT = max(896, -(-(maxsz + 2) // 128) * 128)   # last col of every slot pad
    NCOL = GPC * SLOT
    NTILES = NCOL // 128
    NAH = NCOL // 2                                 # cols per table half
    TROWS = NAH                                     # rows per core per table
    assert NCORES * TROWS < 32768                   # int16 gather indices

    g_of = batch
    core_of_node = g_of // GPC
    newcol = (g_of % GPC) * SLOT + (np.arange(N) - gstart[g_of])

    indeg = np.bincount(dst, minlength=N)
    invdeg_all = (1.0 / np.maximum(indeg, 1.0)).astype(np.float32)

    dstc = core_of_node[dst]
    dcol = newcol[dst]
    tile_of = dcol // 128
    scol = newcol[src]
    score = core_of_node[src]
    half_of = (scol >= NAH).astype(np.int64)
    src_pos = score * TROWS + (scol - half_of * NAH)  # position in half table

    # per (core, half, tile) edge lists
    counts = np.zeros((NCORES, 2, NTILES), dtype=np.int64)
    buckets = {}
    for c in range(NCORES):
        mc = dstc == c
        for h in range(2):
            mh = mc & (half_of == h)
            for t in range(NTILES):
                m = mh & (tile_of == t)
                buckets[(c, h, t)] = (src_pos[m], dcol[m] % 128, src[m])
                counts[c, h, t] = int(m.sum())

    # global block schedule: NBLK[h, t] = max over cores; half A gets >= 1
    # block per tile so the pass-A psum->agg copy always initializes agg cols
    nblk = np.ceil(counts.max(axis=0) / BLK).astype(np.int64)  # [2, NTILES]
    nblk[0] = np.maximum(nblk[0], 1)
    extra = [0, 0]
    for h in range(2):
        tot = int(nblk[h].sum())
        extra[h] = (-tot) % CBLK
    sched = []   # list of (h, t) per block, in execution order
    for h in range(2):
        for t in range(NTILES):
            sched += [(h, t)] * int(nblk[h, t])
        sched += [(h, NTILES - 1)] * extra[h]
    nblk_tot = len(sched)
    nchunks = nblk_tot // CBLK
    assert nchunks * CBLK == nblk_tot
    chunk_half = [sched[k * CBLK][0] for k in range(nchunks)]
    for k in range(nchunks):
        assert all(sched[k * CBLK + j][0] == chunk_half[k] for j in range(CBLK))

    # Real (max-over-cores) block prefix of each (h,t) run: only these blocks
    # are matmul'd, and gather pieces cover exactly their units, so no stale
    # SBUF is ever read.  Pure-padding blocks are skipped entirely.
    run_start = {}
    b0 = 0
    for h in range(2):
        for t in range(NTILES):
            nb = int(nblk[h, t]) + (extra[h] if t == NTILES - 1 else 0)
            run_start[(h, t)] = (b0, nb)
            b0 += nb
    nbr = {}   # real blocks per run (>=1 for half A: initializes agg cols)
    for (h, t), (rb, nb) in run_start.items():
        n = -(-int(counts[:, h, t].max()) // BLK)
        if h == 0:
            n = max(n, 1)
        nbr[(h, t)] = min(n, nb)
    use_block = np.zeros(nblk_tot, dtype=bool)
    blk_start = np.zeros(nblk_tot, dtype=bool)
    blk_stop = np.zeros(nblk_tot, dtype=bool)
    for (h, t), (rb, nb) in run_start.items():
        n = nbr[(h, t)]
        if n > 0:
            use_block[rb:rb + n] = True
            blk_start[rb] = True
            blk_stop[rb + n - 1] = True
    # pieces trimmed to per-run real extents (128-aligned, <=1024 idx each)
    gather_pieces = []
    for k in range(nchunks):
        c0, c1 = k * CBLK * BLK, (k + 1) * CBLK * BLK
        pieces = []
        for (h, t), (rb, nb) in run_start.items():
            if h != chunk_half[k]:
                continue
            s0, s1 = rb * BLK, rb * BLK + nbr[(h, t)] * BLK
            a, b = max(s0, c0), min(s1, c1)
            for s in range(a, b, 1024):
                pieces.append((s - c0, min(1024, b - s)))
        gather_pieces.append(sorted(pieces))
    # chunk after which tile t's aggregation is complete
    last_blk = {}
    for (h, t), (rb, nb) in run_start.items():
        n = nbr[(h, t)]
        if n > 0:
            last_blk[t] = max(last_blk.get(t, -1), rb + n - 1)
    tile_ready = [last_blk[t] // CBLK for t in range(NTILES)]

    # per-core gather idx + per-slot dst columns following the schedule.
    # S blocks are pure one-hot rows, generated ON-CHIP per block via a DVE
    # is_equal against dcol (slot -> dst column, -1 for pad slots); the
    # 1/indeg mean weight is applied later by a diagonal-matmul fused into
    # the per-tile transpose.
    import ml_dtypes
    gidx_cores, dcol_cores, giabs_cores, diag_cores = [], [], [], []
    mask_cores, R2d_cores, S_cores = [], [], []
    inv_cnt = (1.0 / cnt_g.astype(np.float64)).astype(np.float32)
    for c in range(NCORES):
        gi = np.zeros(nblk_tot * BLK, dtype=np.int64)
        ga = np.zeros(nblk_tot * BLK, dtype=np.int64)   # abs src (msgs0)
        dc = np.full(nblk_tot * BLK, -1.0, dtype=np.float32)
        b0 = 0
        for h in range(2):
            for t in range(NTILES):
                nb = int(nblk[h, t]) + (extra[h] if t == NTILES - 1 else 0)
                g, d, a = buckets[(c, h, t)]
                n = len(g)
                gi[b0 * BLK: b0 * BLK + n] = g
                ga[b0 * BLK: b0 * BLK + n] = a
                dc[b0 * BLK: b0 * BLK + n] = d
                b0 += nb
        assert b0 == nblk_tot
        giabs_cores.append(ga)
        gidx_cores.append(_wrap_idx(gi.astype(np.int16), CH))
        # dcol layout [slot-in-block (partition), block]
        dcol_cores.append(np.ascontiguousarray(
            dc.reshape(nblk_tot, BLK).T))
        # dense one-hot S, streamed only by layer 0 (DMA has headroom there;
        # gather-bound layers 1/2 generate S on-chip instead)
        S = np.zeros((nblk_tot * BLK, 128), dtype=np.float32)
        rows = np.arange(nblk_tot * BLK)
        real = dc >= 0
        S[rows[real], dc[real].astype(np.int64)] = 1.0
        S_cores.append(np.ascontiguousarray(
            S.reshape(nblk_tot, BLK, 128).transpose(1, 0, 2)
            .reshape(BLK, nblk_tot * 128)).astype(ml_dtypes.float8_e4m3))

        iv = np.ones(NCOL, np.float32)
        msk = np.zeros(NCOL, np.float32)
        for r in range(GPC):
            g = c * GPC + r
            sz = int(cnt_g[g])
            iv[r * SLOT:r * SLOT + sz] = \
                invdeg_all[gstart[g]:gstart[g] + sz]
            msk[r * SLOT:r * SLOT + sz] = 1.0
        dg = np.zeros((128, NCOL), dtype=np.float32)
        for t in range(NTILES):
            dg[np.arange(128), t * 128 + np.arange(128)] = \
                iv[t * 128:(t + 1) * 128]
        diag_cores.append(dg.astype(ml_dtypes.bfloat16))
        mask_cores.append(
            np.broadcast_to(msk.astype(ml_dtypes.float8_e4m3),
                            (128, NCOL)).copy())

        R2d = np.zeros((2 * GPC, 2 * G), dtype=np.float32)
        for r in range(GPC):
            g = c * GPC + r
            R2d[r, g] = inv_cnt[g]
            R2d[GPC + r, G + g] = 1.0
        R2d_cores.append(R2d)

    return dict(SLOT=SLOT, NCOL=NCOL, NTILES=NTILES, NAH=NAH, TROWS=TROWS,
                cnt_g=cnt_g, gstart=gstart,
                nblk=nblk, extra=extra, sched=sched, nblk_tot=nblk_tot,
                nchunks=nchunks, chunk_half=chunk_half,
                use_block=use_block, blk_start=blk_start, blk_stop=blk_stop,
                gather_pieces=gather_pieces, tile_ready=tile_ready,
                gidx=gidx_cores, giabs=giabs_cores, dcol=dcol_cores,
                S=S_cores,
                diag=diag_cores, mask=mask_cores, R2d=R2d_cores)


# ---------------- device kernel builder --------------------------------------

def _build(nc, pre):
    import concourse.mybir as mybir
    import concourse.tile as tile

    f32 = mybir.dt.float32
    f16 = mybir.dt.float16
    bf16 = mybir.dt.bfloat16
    f8 = mybir.dt.float8e4
    i16 = mybir.dt.int16
    NCH = pre['nchunks']
    NBLK_TOT = pre['nblk_tot']
    NCOL, NTILES, NAH, SLOT = \
        pre['NCOL'], pre['NTILES'], pre['NAH'], pre['SLOT']
    TROWS = pre['TROWS']
    HROWS = NCORES * TROWS      # rows per (A or B) gather table
    HT = NTILES // 2            # tiles per half
    SGRP = SLOT // 128          # tiles per bounce-ship DMA (= tiles/slot)
    NGRP = HT // SGRP           # = GPC // 2
    assert NGRP * SGRP == HT
    sched = pre['sched']

    use_block = pre['use_block']
    is_start = pre['blk_start']
    is_stop = pre['blk_stop']

    # ---- I/O ----
    msgs0_d = nc.dram_tensor("msgs0", [NCH, 128, CBLK * D], f16,
                             kind="ExternalInput")
    xownT = nc.dram_tensor("xownT", [D, NCOL], f32, kind="ExternalInput")
    gidx_d = nc.dram_tensor("gidx", [NCH, 128, CPW], i16, kind="ExternalInput")
    S_d = nc.dram_tensor("S", [BLK, NBLK_TOT * 128], f8, kind="ExternalInput")
    dcol_d = nc.dram_tensor("dcol", [BLK, NBLK_TOT], f32, kind="ExternalInput")
    colidx_d = nc.dram_tensor("colidx", [128, 128], f32, kind="ExternalInput")
    diag_d = nc.dram_tensor("diag", [128, NCOL], bf16, kind="ExternalInput")
    mask_d = nc.dram_tensor("mask", [128, NCOL], f8, kind="ExternalInput")
    R2d_d = nc.dram_tensor("R2d", [2 * GPC, 2 * G], f32, kind="ExternalInput")
    gfT_d = nc.dram_tensor("gfT", [G_FEAT, G], f32, kind="ExternalInput")
    ident_d = nc.dram_tensor("ident", [128, 128], f32, kind="ExternalInput")
    Wl_d = [nc.dram_tensor(f"Wl{i}", [D, HID], f32, kind="ExternalInput")
            for i in range(3)]
    Wr_d = [nc.dram_tensor(f"Wr{i}", [D, HID], f32, kind="ExternalInput")
            for i in range(3)]
    gb_d = [nc.dram_tensor(f"gb{i}", [HID, 2], f32, kind="ExternalInput")
            for i in range(3)]
    W1_d = nc.dram_tensor("W1", [2 * HID + G_FEAT, HID], f32, kind="ExternalInput")
    W2_d = nc.dram_tensor("W2", [HID, HID // 2], f32, kind="ExternalInput")
    W3_d = nc.dram_tensor("W3", [HID // 2, 1], f32, kind="ExternalInput")
    bT_d = nc.dram_tensor("bT", [HID, 3], f32, kind="ExternalInput")

    out_d = nc.dram_tensor("out", [G], f32, kind="ExternalOutput")

    rg = [list(range(NCORES))]

    with tile.TileContext(nc) as tc:
        with (
            tc.tile_pool(name="sb", bufs=3) as sb,
            tc.tile_pool(name="big", bufs=2) as bigp,       # zT (f32 NCOL)
            tc.tile_pool(name="agg", bufs=1) as aggp,       # agg_sb bf16
            tc.tile_pool(name="big1", bufs=1) as big1,      # allp
            tc.tile_pool(name="msg", bufs=3) as msgp,       # msgs + S per chunk
            tc.tile_pool(name="idx", bufs=6) as idxp,
            tc.tile_pool(name="stg", bufs=2) as stgp,       # bounce staging
            tc.tile_pool(name="cst", bufs=1) as cst,
            tc.tile_pool(name="ps", bufs=2, space="PSUM") as ps,
            tc.tile_pool(name="pst", bufs=2, space="PSUM") as pst,
            tc.tile_pool(name="psa", bufs=2, space="PSUM") as psa,
            tc.tile_pool(name="psm", bufs=1, space="PSUM") as psm,
            tc.tile_pool(name="dram", bufs=1, space="DRAM") as dram,
        ):
            # ---- DRAM scratch ----
            hbA = [dram.tile([TROWS, D], f16, tag=f"hbA{i}", name=f"hbA{i}")
                   for i in range(2)]
            hbB = [dram.tile([TROWS, D], f16, tag=f"hbB{i}", name=f"hbB{i}")
                   for i in range(2)]
            tblA = [dram.tile([HROWS, D], f16, tag=f"tblA{i}",
                              name=f"tblA{i}", addr_space="Shared")
                    for i in range(2)]
            tblB = [dram.tile([HROWS, D], f16, tag=f"tblB{i}",
                              name=f"tblB{i}", addr_space="Shared")
                    for i in range(2)]
            stats_in = [dram.tile([D, 2], f32, tag=f"stats_in{i}",
                                  name=f"stats_in{i}") for i in range(3)]
            stats_out = [dram.tile([NCORES * D, 2], f32, tag=f"stats_out{i}",
                                   name=f"stats_out{i}", addr_space="Shared")
                         for i in range(3)]
            pool_in = dram.tile([D, 2 * G], f32, tag="pool_in")
            pool_out = dram.tile([NCORES * D, 2 * G], f32, tag="pool_out",
                                 addr_space="Shared")

            def load_const(src_ap, rows, cols, name, dt=f32):
                t = cst.tile([rows, cols], dt, tag=name)
                nc.sync.dma_start(out=t[:, :], in_=src_ap)
                return t

            ident_sb = load_const(ident_d[:, :], 128, 128, "ident")
            diag_sb = load_const(diag_d[:, :], 128, NCOL, "diag", bf16)
            mask_sb = load_const(mask_d[:, :], 128, NCOL, "mask", f8)
            dcol_sb = load_const(dcol_d[:, :], BLK, NBLK_TOT, "dcol")
            colidx_sb = load_const(colidx_d[:, :], 128, 128, "colidx")
            xT_sb = bigp.tile([128, NCOL], f32, tag="zT")
            nc.sync.dma_start(out=xT_sb[:, :], in_=xownT[:, :])

            hT_prev = xT_sb

            # deferred AG_B trigger (emitted inside the NEXT layer's chunk
            # loop so A-chunk gathers overlap the B AllGather wire time)
            pending_agb = [None]

            def emit_agb():
                if pending_agb[0] is not None:
                    hb, tb = pending_agb[0]
                    nc.gpsimd.collective_compute(
                        "AllGather", mybir.AluOpType.bypass,
                        replica_groups=rg, ins=[hb.opt()], outs=[tb.opt()])
                    pending_agb[0] = None

            for li in range(3):
                Wl_sb = load_const(Wl_d[li][:, :], D, HID, f"Wl{li}")
                Wr_sb = load_const(Wr_d[li][:, :], D, HID, f"Wr{li}")
                gb_sb = load_const(gb_d[li][:, :], HID, 2, f"gb{li}")

                agg_sb = aggp.tile([128, NCOL], bf16, tag="agg")
                zT = bigp.tile([128, NCOL], f32, tag="zT")
                zsum = sb.tile([128, NTILES], f32, tag="zsum")
                zsq = sb.tile([128, NTILES], f32, tag="zsq")
                sq_scr = sb.tile([128, D], f32, tag="sqscr")

                def dense_tile(t, zT=zT, zsum=zsum, zsq=zsq, sq_scr=sq_scr,
                               agg_sb=agg_sb, Wl_sb=Wl_sb, Wr_sb=Wr_sb,
                               hT_prev=hT_prev):
                    aT_ps = pst.tile([128, D], f32, tag="tp")
                    # transpose fused with the 1/indeg column scale:
                    # out[f, j] = sum_k agg[k, f] * diag[k, j] = agg[j, f]/deg_j
                    nc.tensor.matmul(aT_ps[:, :],
                                     agg_sb[:, t * 128:(t + 1) * 128],
                                     diag_sb[:, t * 128:(t + 1) * 128],
                                     start=True, stop=True)
                    aT_sb = sb.tile([128, D], f32, tag="aTs")
                    nc.vector.tensor_copy(aT_sb[:, :], aT_ps[:, :])
                    z_ps = ps.tile([128, D], f32, tag="z")
                    nc.tensor.matmul(z_ps[:, :], Wl_sb[:, :], aT_sb[:, :],
                                     start=True, stop=False)
                    nc.tensor.matmul(z_ps[:, :], Wr_sb[:, :],
                                     hT_prev[:, t * 128:(t + 1) * 128],
                                     start=False, stop=True)
                    nc.scalar.activation(zT[:, t * 128:(t + 1) * 128],
                                         z_ps[:, :],
                                         mybir.ActivationFunctionType.Copy,
                                         accum_out=zsum[:, t:t + 1])
                    nc.scalar.activation(sq_scr[:, :],
                                         zT[:, t * 128:(t + 1) * 128],
                                         mybir.ActivationFunctionType.Square,
                                         accum_out=zsq[:, t:t + 1])

                # ---- gather + S-matmul aggregation, dense tiles interleaved
                # into the chunk loop as their aggregation completes (engine
                # queues execute in emission order - this is what overlaps
                # the dense phase with the gather pipeline) ----
                acc_ps = None
                gq = [0]
                tdone = 0
                for k in range(NCH):
                    h = pre['chunk_half'][k]
                    if li > 0 and h == 1:
                        emit_agb()   # half-B gathers need the B AllGather
                    msgs = msgp.tile([128, CBLK, D], f16, tag="msgs")
                    if li == 0:
                        nc.sync.dma_start(
                            out=msgs[:, :, :],
                            in_=msgs0_d[k, :, :].rearrange(
                                "p (j d) -> p j d", d=D))
                    else:
                        src_tab = (tblA if h == 0 else tblB)[(li - 1) % 2]
                        gi_t = idxp.tile([128, CPW], i16, tag="gi")
                        nc.sync.dma_start(out=gi_t[:], in_=gidx_d[k, :, :])
                        # runtime faults on dma_gather num_idxs > 1024:
                        # <=1024-index sub-gathers trimmed to per-run real
                        # extents; round-robin the 4 SWDGE queues
                        for off, n in pre['gather_pieces'][k]:
                            nc.gpsimd.dma_gather(
                                msgs[:, off // 128:off // 128
                                     + (n + 127) // 128, :],
                                src_tab[0:HROWS, :],
                                gi_t[:, off // 16:off // 16 + n // 16],
                                n, n, D, queue_num=gq[0] % NQ)
                            gq[0] += 1
                    # S blocks are one-hot.  Layer 0 streams them (its DMA
                    # has headroom); gather-bound layers 1/2 generate them
                    # on-chip (DVE is_equal of the column-iota against dcol)
                    S_t = msgp.tile([128, CBLK, D], f8, tag="Ssb")
                    if li == 0:
                        nc.scalar.dma_start(
                            out=S_t[:, :, :],
                            in_=S_d[:, k * CBLK * 128:(k + 1) * CBLK * 128]
                            .rearrange("p (j d) -> p j d", d=128))
                    else:
                        for j in range(CBLK):
                            b = k * CBLK + j
                            if not use_block[b]:
                                continue
                            nc.vector.scalar_tensor_tensor(
                                S_t[:, j, :], colidx_sb[:, :],
                                dcol_sb[:, b:b + 1], colidx_sb[:, :],
                                mybir.AluOpType.is_equal,
                                mybir.AluOpType.bypass)
                    for j in range(CBLK):
                        b = k * CBLK + j
                        if not use_block[b]:
                            continue
                        h_b, t_b = sched[b]
                        if is_start[b]:
                            acc_ps = psa.tile([128, D], f32, tag="accp")
                        nc.tensor.matmul(acc_ps[:, :], S_t[:, j, :],
                                         msgs[:, j, :],
                                         start=is_start[b], stop=is_stop[b])
                        if is_stop[b]:
                            if h_b == 0:
                                nc.scalar.copy(
                                    agg_sb[:, t_b * 128:(t_b + 1) * 128],
                                    acc_ps[:, :])
                            else:
                                nc.vector.tensor_add(
                                    agg_sb[:, t_b * 128:(t_b + 1) * 128],
                                    agg_sb[:, t_b * 128:(t_b + 1) * 128],
                                    acc_ps[:, :])
                    if li > 0 and k == TRIG_B_AFTER:
                        emit_agb()
                    while tdone < NTILES and pre['tile_ready'][tdone] <= k:
                        dense_tile(tdone)
                        tdone += 1
                emit_agb()
                while tdone < NTILES:
                    dense_tile(tdone)
                    tdone += 1

                stat_sb = sb.tile([128, 2], f32, tag="stat")
                nc.vector.tensor_reduce(stat_sb[:, 0:1], zsum[:, :],
                                        mybir.AxisListType.X,
                                        mybir.AluOpType.add)
                nc.vector.tensor_reduce(stat_sb[:, 1:2], zsq[:, :],
                                        mybir.AxisListType.X,
                                        mybir.AluOpType.add)
                nc.sync.dma_start(out=stats_in[li][:, :], in_=stat_sb[:, :])
                nc.gpsimd.collective_compute(
                    "AllGather", mybir.AluOpType.bypass, replica_groups=rg,
                    ins=[stats_in[li].opt()], outs=[stats_out[li].opt()])
                allst = sb.tile([128, NCORES, 2], f32, tag="allst")
                nc.sync.dma_start(
                    out=allst[:, :, :],
                    in_=stats_out[li][:, :].rearrange("(c p) j -> p c j",
                                                      c=NCORES))
                tot = sb.tile([128, 2], f32, tag="tot")
                nc.vector.tensor_add(tot[:, :], allst[:, 0, :], allst[:, 1, :])
                for c in range(2, NCORES):
                    nc.vector.tensor_add(tot[:, :], tot[:, :], allst[:, c, :])
                mu = sb.tile([128, 6], f32, tag="mu")
                nc.scalar.mul(mu[:, 0:1], tot[:, 0:1], 1.0 / N)
                nc.scalar.mul(mu[:, 1:2], tot[:, 1:2], 1.0 / N)
                nc.vector.tensor_mul(mu[:, 2:3], mu[:, 0:1], mu[:, 0:1])
                nc.vector.tensor_sub(mu[:, 3:4], mu[:, 1:2], mu[:, 2:3])
                nc.vector.tensor_scalar_add(mu[:, 3:4], mu[:, 3:4], EPS)
                nc.vector.reciprocal(mu[:, 4:5], mu[:, 3:4])
                nc.scalar.sqrt(mu[:, 4:5], mu[:, 4:5])
                nc.vector.tensor_mul(mu[:, 4:5], mu[:, 4:5], gb_sb[:, 0:1])
                nc.vector.tensor_mul(mu[:, 5:6], mu[:, 0:1], mu[:, 4:5])
                nc.vector.tensor_sub(mu[:, 5:6], gb_sb[:, 1:2], mu[:, 5:6])
                # h = relu(z*s + shift), pads zeroed via the fp8 column mask
                # (pads must stay 0 for BN stats + pooling), interleaved
                # per-tile with the transpose-ship / pooling consumers
                def apply_tile(t, zT=zT, mu=mu):
                    sl = slice(t * 128, (t + 1) * 128)
                    nc.scalar.activation(zT[:, sl], zT[:, sl],
                                         mybir.ActivationFunctionType.Relu,
                                         bias=mu[:, 5:6], scale=mu[:, 4:5])
                    nc.vector.tensor_mul(zT[:, sl], zT[:, sl], mask_sb[:, sl])
                hT_prev = zT

                if li < 2:
                    # ---- transpose back; ship f16 halves to split AG ----
                    for half, hb, tb in ((0, hbA[li % 2], tblA[li % 2]),
                                         (1, hbB[li % 2], tblB[li % 2])):
                        for grp in range(NGRP):
                            stg = stgp.tile([128, SGRP, 128], f16, tag="stg")
                            for j in range(SGRP):
                                t = half * HT + grp * SGRP + j
                                apply_tile(t)
                                hT_ps = pst.tile([128, D], f32, tag="tp")
                                nc.tensor.transpose(
                                    hT_ps[:, :],
                                    zT[:, t * 128:(t + 1) * 128],
                                    ident_sb[:, :])
                                nc.vector.tensor_copy(stg[:, j, :],
                                                      hT_ps[:, :])
                            r0 = grp * SGRP * 128
                            nc.sync.dma_start(
                                out=hb[r0:r0 + SGRP * 128, :].rearrange(
                                    "(j p) d -> p j d", p=128),
                                in_=stg[:, :, :])
                        if half == 0:
                            nc.gpsimd.collective_compute(
                                "AllGather", mybir.AluOpType.bypass,
                                replica_groups=rg,
                                ins=[hb.opt()], outs=[tb.opt()])
                        else:
                            pending_agb[0] = (hb, tb)
                else:
                    # ---- pooling: uniform per-slot column reduces ----
                    loc = sb.tile([128, 2 * GPC], f32, tag="loc")
                    for r in range(GPC):
                        for j in range(SGRP):
                            apply_tile(r * SGRP + j)
                        sl = slice(r * SLOT, (r + 1) * SLOT)
                        nc.vector.tensor_reduce(loc[:, r:r + 1], zT[:, sl],
                                                mybir.AxisListType.X,
                                                mybir.AluOpType.add)
                        nc.vector.tensor_reduce(loc[:, GPC + r:GPC + r + 1],
                                                zT[:, sl],
                                                mybir.AxisListType.X,
                                                mybir.AluOpType.max)
                    locT_ps = pst.tile([2 * GPC, 128], f32, tag="tp",
                                       name="locT")
                    nc.tensor.transpose(locT_ps[:, :], loc[:, :],
                                        ident_sb[:, :])
                    locT_sb = sb.tile([2 * GPC, 128], f32, tag="locTs")
                    nc.vector.tensor_copy(locT_sb[:, :], locT_ps[:, :])
                    R2d_sb = load_const(R2d_d[:, :], 2 * GPC, 2 * G, "R2d")
                    pool_ps = psm.tile([128, 2 * G], f32, tag="tail")
                    nc.tensor.matmul(pool_ps[:, :], locT_sb[:, :],
                                     R2d_sb[:, :], start=True, stop=True)

            # ---- pool partial exchange ----
            pool_sb = sb.tile([128, 2 * G], f32, tag="poolp")
            nc.vector.tensor_copy(pool_sb[:, :], pool_ps[:, :])
            nc.sync.dma_start(out=pool_in[:, :], in_=pool_sb[:, :])
            nc.gpsimd.collective_compute(
                "AllGather", mybir.AluOpType.bypass, replica_groups=rg,
                ins=[pool_in.opt()], outs=[pool_out.opt()])
            allp = big1.tile([128, NCORES, 2 * G], f32, tag="allp")
            nc.sync.dma_start(
                out=allp[:, :, :],
                in_=pool_out[:, :].rearrange("(c p) j -> p c j", c=NCORES))
            meanTot = sb.tile([128, G], f32, tag="meanTot")
            maxTot = sb.tile([128, G], f32, tag="maxTot")
            nc.vector.tensor_add(meanTot[:, :], allp[:, 0, 0:G],
                                 allp[:, 1, 0:G])
            nc.vector.tensor_max(maxTot[:, :], allp[:, 0, G:2 * G],
                                 allp[:, 1, G:2 * G])
            for c in range(2, NCORES):
                nc.vector.tensor_add(meanTot[:, :], meanTot[:, :],
                                     allp[:, c, 0:G])
                nc.vector.tensor_max(maxTot[:, :], maxTot[:, :],
                                     allp[:, c, G:2 * G])

            # ---- head (feature-major) ----
            W1a_sb = load_const(W1_d[0:HID, :], HID, HID, "W1a")
            W1b_sb = load_const(W1_d[HID:2 * HID, :], HID, HID, "W1b")
            W1c_sb = load_const(W1_d[2 * HID:2 * HID + G_FEAT, :], G_FEAT,
                                HID, "W1c")
            W2_sb = load_const(W2_d[:, :], HID, HID // 2, "W2")
            W3_sb = load_const(W3_d[:, :], HID // 2, 1, "W3")
            bT_sb = load_const(bT_d[:, :], HID, 3, "bT")
            gfT_sb = load_const(gfT_d[:, :], G_FEAT, G, "gfT")

            m1_ps = psm.tile([HID, G], f32, tag="tail")
            nc.tensor.matmul(m1_ps[:, :], W1a_sb[:, :], meanTot[:, :],
                             start=True, stop=False)
            nc.tensor.matmul(m1_ps[:, :], W1b_sb[:, :], maxTot[:, :],
                             start=False, stop=False)
            nc.tensor.matmul(m1_ps[:, :], W1c_sb[:, :],
                             gfT_sb[:, :], start=False, stop=True)
            m1_sb = sb.tile([HID, G], f32, tag="m1s")
            nc.scalar.activation(m1_sb[:, :], m1_ps[:, :],
                                 mybir.ActivationFunctionType.Relu,
                                 bias=bT_sb[:, 0:1])
            m2_ps = psm.tile([HID // 2, G], f32, tag="tail")
            nc.tensor.matmul(m2_ps[:, :], W2_sb[:, :], m1_sb[:, :],
                             start=True, stop=True)
            m2_sb = sb.tile([HID // 2, G], f32, tag="m2s")
            nc.scalar.activation(m2_sb[:, :], m2_ps[:, :],
                                 mybir.ActivationFunctionType.Relu,
                                 bias=bT_sb[0:HID // 2, 1:2])
            m3_ps = psm.tile([1, G], f32, tag="tail")
            nc.tensor.matmul(m3_ps[:, :], W3_sb[:, :], m2_sb[:, :],
                             start=True, stop=True)
            m3_sb = sb.tile([1, G], f32, tag="m3s")
            nc.scalar.copy(m3_sb[:, :], m3_ps[:, :])
            nc.vector.tensor_scalar_add(m3_sb[:, :], m3_sb[:, :],
                                        bT_sb[0:1, 2:3])
            nc.sync.dma_start(out=out_d[:].rearrange("(o g) -> o g", o=1),
                              in_=m3_sb[:, :])
    return nc


# ---------------- public entry ------------------------------------------------

def build_in_maps(x, edge_index, batch, g_feats, params, pre):
    x = np.asarray(x, dtype=np.float32)
    g_feats = np.asarray(g_feats, dtype=np.float32)
    batch = np.asarray(batch, dtype=np.int64)

    bT = np.zeros((HID, 3), np.float32)
    bT[:, 0] = np.asarray(params['b1'], np.float32)
    bT[:HID // 2, 1] = np.asarray(params['b2'], np.float32)
    bT[0, 2] = np.asarray(params['b3'], np.float32).reshape(-1)[0]

    common = {
        "ident": np.eye(128, dtype=np.float32),
        "colidx": np.broadcast_to(
            np.arange(128, dtype=np.float32), (128, 128)).copy(),
        "gfT": np.ascontiguousarray(g_feats.T),
        "W1": np.asarray(params['W1'], np.float32),
        "W2": np.asarray(params['W2'], np.float32),
        "W3": np.asarray(params['W3'], np.float32),
        "bT": bT,
    }
    for i in range(3):
        common[f"Wl{i}"] = np.asarray(params[f'Wl{i}'], np.float32)
        common[f"Wr{i}"] = np.asarray(params[f'Wr{i}'], np.float32)
        gb = np.zeros((HID, 2), np.float32)
        gb[:, 0] = np.asarray(params[f'gamma{i}'], np.float32)
        gb[:, 1] = np.asarray(params[f'beta{i}'], np.float32)
        common[f"gb{i}"] = gb

    x16 = x.astype(np.float16)
    NCH = pre['nchunks']
    SLOT, NCOL = pre['SLOT'], pre['NCOL']
    cnt_g, gstart = pre['cnt_g'], pre['gstart']
    in_maps = []
    for c in range(NCORES):
        xo = np.zeros((NCOL, D), np.float32)
        for r in range(GPC):
            g = c * GPC + r
            sz = int(cnt_g[g])
            xo[r * SLOT:r * SLOT + sz] = x[gstart[g]:gstart[g] + sz]
        # pre-gather layer-0 messages into the exact chunk SBUF layout:
        # slot s of chunk k -> partition s%128, free block s//128
        gi_abs = pre['giabs'][c]
        msgs0 = x16[gi_abs].reshape(NCH, CBLK, BLK, D).transpose(0, 2, 1, 3)
        msgs0 = np.ascontiguousarray(msgs0.reshape(NCH, 128, CBLK * D))
        m = dict(common)
        m.update({
            "xownT": np.ascontiguousarray(xo.T),
            "msgs0": msgs0,
            "gidx": pre['gidx'][c],
            "S": pre['S'][c],
            "dcol": pre['dcol'][c],
            "diag": pre['diag'][c],
            "mask": pre['mask'][c],
            "R2d": pre['R2d'][c],
        })
        in_maps.append(m)
    return in_maps


def build_nc(pre):
    import os
    import concourse.bacc as bacc
    nc = bacc.Bacc(None, target_bir_lowering=False, debug=False,
                   num_devices=NCORES, num_swdge_queues=4,
                   detect_race_conditions=os.environ.get(
                       "KERNEL_NO_RACE_CHECK") != "1")
    nc = _build(nc, pre)
    nc.compile()
    return nc


def kernel(x, edge_index, batch, g_feats,
           Wl0, bl0, Wr0, gamma0, beta0,
           Wl1, bl1, Wr1, gamma1, beta1,
           Wl2, bl2, Wr2, gamma2, beta2,
           W1, b1, W2, b2, W3, b3):
    # bl{i} cancels inside BatchNorm (constant pre-BN shift), so it is unused.
    from concourse.bass_utils import run_bass_kernel_spmd

    params = dict(Wl0=Wl0, Wr0=Wr0, gamma0=gamma0, beta0=beta0,
                  Wl1=Wl1, Wr1=Wr1, gamma1=gamma1, beta1=beta1,
                  Wl2=Wl2, Wr2=Wr2, gamma2=gamma2, beta2=beta2,
                  W1=W1, b1=b1, W2=W2, b2=b2, W3=W3, b3=b3)
    pre = _preprocess(x, edge_index, batch)
    nc = build_nc(pre)
    in_maps = build_in_maps(x, edge_index, batch, g_feats, params, pre)
    res = run_bass_kernel_spmd(nc, in_maps, list(range(NCORES)))
    return np.asarray(res.results[0]["out"], dtype=np.float32)


# revision 36
# speedup vs baseline: 1.1097x; 1.0372x over previous
"""Distributed Bass/Trainium2 kernel for nn_AreaGNN: 3x SAGEConv(mean) +
global BatchNorm + ReLU, per-graph mean/max pooling, 3-layer MLP head.
SPMD across 8 NeuronCores; takes FULL inputs, returns FULL output [G].

Node layout: batch is sorted, so graphs are contiguous; core c owns graphs
8c..8c+7, each in a fixed SLOT-column window of the feature-major zT (pads
zeroed by a per-core fp8 column mask after each ReLU).  Pooling is then just
uniform per-slot column reduces + one tiny route matmul - no transposes, no
slot gather.  Edges are owned by the dst graph's core, sorted by (src-half,
dst-tile), padded into 128-edge blocks with a global SPMD-uniform schedule.
Layer-0 messages are pre-gathered host-side and streamed; layer-1/2 messages
come from split (A/B) f16 AllGather node tables via hardware dma_gather over
4 SWDGE queues.  The A-half AllGather is issued mid-transpose-loop and the
B-half trigger is deferred into the next layer's chunk loop so gathers for
A-chunks overlap the B AllGather wire time.  Segment sums run on the
TensorEngine via per-block one-hot S matrices (fp8) accumulated in f32 PSUM;
the 1/indeg mean weighting is folded into the per-tile transpose as a
bf16 diagonal matmul.
"""
import numpy as np

N = 50000
E = 800000
D = 128
HID = 128
G = 64
G_FEAT = 32
EPS = 1e-5
NCORES = 8
GPC = G // NCORES           # 8 graphs per core
BLK = 128                   # edges per S block
CBLK = 32                   # blocks per gather chunk (4096 edges)
CH = BLK * CBLK
CPW = CH // 16
NQ = 4                      # SWDGE queues used for gathers
TRIG_B_AFTER = 6            # chunks of next layer emitted before AG_B trigger


# ---------------- host-side preprocessing -----------------------------------

def _wrap_idx(idx, ch):
    """[L] -> [L/ch, 128, ch/16] int16: element m of a chunk at (m%16, m//16),
    replicated across the eight 16-partition groups."""
    L = idx.shape[0]
    out = np.empty((L // ch, 128, ch // 16), dtype=np.int16)
    w = idx.reshape(L // ch, ch // 16, 16).transpose(0, 2, 1)
    for g in range(8):
        out[:, g * 16:(g + 1) * 16, :] = w
    return out


def _preprocess(x, edge_index, batch):
    src = np.asarray(edge_index[0], dtype=np.int64)
    dst = np.asarray(edge_index[1], dtype=np.int64)
    batch = np.asarray(batch, dtype=np.int64)

    # ---- node layout: graph g -> core g//GPC, slot g%GPC ----
    cnt_g = np.bincount(batch, minlength=G)
    assert cnt_g.min() >= 1, "empty graph unsupported"
    gstart = np.concatenate([[0], np.cumsum(cnt_g)])
    maxsz = int(cnt_g.max())
    SLOT = max(896, -(-(maxsz + 2) // 128) * 128)   # last col of every slot pad
    NCOL = GPC * SLOT
    NTILES = NCOL // 128
    NAH = NCOL // 2                                 # cols per table half
    TROWS = NAH                                     # rows per core per table
    assert NCORES * TROWS < 32768                   # int16 gather indices

    g_of = batch
    core_of_node = g_of // GPC
    newcol = (g_of % GPC) * SLOT + (np.arange(N) - gstart[g_of])

    indeg = np.bincount(dst, minlength=N)
    invdeg_all = (1.0 / np.maximum(indeg, 1.0)).astype(np.float32)

    dstc = core_of_node[dst]
    dcol = newcol[dst]
    tile_of = dcol // 128
    scol = newcol[src]
    score = core_of_node[src]
    half_of = (scol >= NAH).astype(np.int64)
    src_pos = score * TROWS + (scol - half_of * NAH)  # position in half table

    # per (core, half, tile) edge lists
    counts = np.zeros((NCORES, 2, NTILES), dtype=np.int64)
    buckets = {}
    for c in range(NCORES):
        mc = dstc == c
        for h in range(2):
            mh = mc & (half_of == h)
            for t in range(NTILES):
                m = mh & (tile_of == t)
                buckets[(c, h, t)] = (src_pos[m], dcol[m] % 128, src[m])
                counts[c, h, t] = int(m.sum())

    # global block schedule: NBLK[h, t] = max over cores; half A gets >= 1
    # block per tile so the pass-A psum->agg copy always initializes agg cols
    nblk = np.ceil(counts.max(axis=0) / BLK).astype(np.int64)  # [2, NTILES]
    nblk[0] = np.maximum(nblk[0], 1)
    extra = [0, 0]
    for h in range(2):
        tot = int(nblk[h].sum())
        extra[h] = (-tot) % CBLK
    sched = []   # list of (h, t) per block, in execution order
    for h in range(2):
        for t in range(NTILES):
            sched += [(h, t)] * int(nblk[h, t])
        sched += [(h, NTILES - 1)] * extra[h]
    nblk_tot = len(sched)
    nchunks = nblk_tot // CBLK
    assert nchunks * CBLK == nblk_tot
    chunk_half = [sched[k * CBLK][0] for k in range(nchunks)]
    for k in range(nchunks):
        assert all(sched[k * CBLK + j][0] == chunk_half[k] for j in range(CBLK))

    # Real (max-over-cores) block prefix of each (h,t) run: only these blocks
    # are matmul'd, and gather pieces cover exactly their units, so no stale
    # SBUF is ever read.  Pure-padding blocks are skipped entirely.
    run_start = {}
    b0 = 0
    for h in range(2):
        for t in range(NTILES):
            nb = int(nblk[h, t]) + (extra[h] if t == NTILES - 1 else 0)
            run_start[(h, t)] = (b0, nb)
            b0 += nb
    nbr = {}   # real blocks per run (>=1 for half A: initializes agg cols)
    for (h, t), (rb, nb) in run_start.items():
        n = -(-int(counts[:, h, t].max()) // BLK)
        if h == 0:
            n = max(n, 1)
        nbr[(h, t)] = min(n, nb)
    use_block = np.zeros(nblk_tot, dtype=bool)
    blk_start = np.zeros(nblk_tot, dtype=bool)
    blk_stop = np.zeros(nblk_tot, dtype=bool)
    for (h, t), (rb, nb) in run_start.items():
        n = nbr[(h, t)]
        if n > 0:
            use_block[rb:rb + n] = True
            blk_start[rb] = True
            blk_stop[rb + n - 1] = True
    # pieces trimmed to per-run real extents (128-aligned, <=1024 idx each)
    gather_pieces = []
    for k in range(nchunks):
        c0, c1 = k * CBLK * BLK, (k + 1) * CBLK * BLK
        pieces = []
        for (h, t), (rb, nb) in run_start.items():
            if h != chunk_half[k]:
                continue
            s0, s1 = rb * BLK, rb * BLK + nbr[(h, t)] * BLK
            a, b = max(s0, c0), min(s1, c1)
            for s in range(a, b, 1024):
                pieces.append((s - c0, min(1024, b - s)))
        gather_pieces.append(sorted(pieces))
    # chunk after which tile t's aggregation is complete
    last_blk = {}
    for (h, t), (rb, nb) in run_start.items():
        n = nbr[(h, t)]
        if n > 0:
            last_blk[t] = max(last_blk.get(t, -1), rb + n - 1)
    tile_ready = [last_blk[t] // CBLK for t in range(NTILES)]

    # per-core gather idx + per-slot dst columns following the schedule.
    # S blocks are pure one-hot rows, generated ON-CHIP per block via a DVE
    # is_equal against dcol (slot -> dst column, -1 for pad slots); the
    # 1/indeg mean weight is applied later by a diagonal-matmul fused into
    # the per-tile transpose.
    import ml_dtypes
    gidx_cores, dcol_cores, giabs_cores, diag_cores = [], [], [], []
    mask_cores, R2d_cores, S_cores = [], [], []
    inv_cnt = (1.0 / cnt_g.astype(np.float64)).astype(np.float32)
    for c in range(NCORES):
        gi = np.zeros(nblk_tot * BLK, dtype=np.int64)
        ga = np.zeros(nblk_tot * BLK, dtype=np.int64)   # abs src (msgs0)
        dc = np.full(nblk_tot * BLK, -1.0, dtype=np.float32)
        b0 = 0
        for h in range(2):
            for t in range(NTILES):
                nb = int(nblk[h, t]) + (extra[h] if t == NTILES - 1 else 0)
                g, d, a = buckets[(c, h, t)]
                n = len(g)
                gi[b0 * BLK: b0 * BLK + n] = g
                ga[b0 * BLK: b0 * BLK + n] = a
                dc[b0 * BLK: b0 * BLK + n] = d
                b0 += nb
        assert b0 == nblk_tot
        giabs_cores.append(ga)
        gidx_cores.append(_wrap_idx(gi.astype(np.int16), CH))
        # dcol layout [slot-in-block (partition), block]
        dcol_cores.append(np.ascontiguousarray(
            dc.reshape(nblk_tot, BLK).T))
        # dense one-hot S, streamed only by layer 0 (DMA has headroom there;
        # gather-bound layers 1/2 generate S on-chip instead)
        S = np.zeros((nblk_tot * BLK, 128), dtype=np.float32)
        rows = np.arange(nblk_tot * BLK)
        real = dc >= 0
        S[rows[real], dc[real].astype(np.int64)] = 1.0
        S_cores.append(np.ascontiguousarray(
            S.reshape(nblk_tot, BLK, 128).transpose(1, 0, 2)
            .reshape(BLK, nblk_tot * 128)).astype(ml_dtypes.float8_e4m3))

        iv = np.ones(NCOL, np.float32)
        msk = np.zeros(NCOL, np.float32)
        for r in range(GPC):
            g = c * GPC + r
            sz = int(cnt_g[g])
            iv[r * SLOT:r * SLOT + sz] = \
                invdeg_all[gstart[g]:gstart[g] + sz]
            msk[r * SLOT:r * SLOT + sz] = 1.0
        dg = np.zeros((128, NCOL), dtype=np.float32)
        for t in range(NTILES):
            dg[np.arange(128), t * 128 + np.arange(128)] = \
                iv[t * 128:(t + 1) * 128]
        diag_cores.append(dg.astype(ml_dtypes.bfloat16))
        mask_cores.append(
            np.broadcast_to(msk.astype(ml_dtypes.float8_e4m3),
                            (128, NCOL)).copy())

        R2d = np.zeros((2 * GPC, 2 * G), dtype=np.float32)
        for r in range(GPC):
            g = c * GPC + r
            R2d[r, g] = inv_cnt[g]
            R2d[GPC + r, G + g] = 1.0
        R2d_cores.append(R2d)

    return dict(SLOT=SLOT, NCOL=NCOL, NTILES=NTILES, NAH=NAH, TROWS=TROWS,
                cnt_g=cnt_g, gstart=gstart,
                nblk=nblk, extra=extra, sched=sched, nblk_tot=nblk_tot,
                nchunks=nchunks, chunk_half=chunk_half,
                use_block=use_block, blk_start=blk_start, blk_stop=blk_stop,
                gather_pieces=gather_pieces, tile_ready=tile_ready,
                gidx=gidx_cores, giabs=giabs_cores, dcol=dcol_cores,
                S=S_cores,
                diag=diag_cores, mask=mask_cores, R2d=R2d_cores)


# ---------------- device kernel builder --------------------------------------

def _build(nc, pre):
    import concourse.mybir as mybir
    import concourse.tile as tile

    f32 = mybir.dt.float32
    f16 = mybir.dt.float16
    bf16 = mybir.dt.bfloat16
    f8 = mybir.dt.float8e4
    i16 = mybir.dt.int16
    i8 = mybir.dt.int8
    NCH = pre['nchunks']
    NBLK_TOT = pre['nblk_tot']
    NCOL, NTILES, NAH, SLOT = \
        pre['NCOL'], pre['NTILES'], pre['NAH'], pre['SLOT']
    TROWS = pre['TROWS']
    HROWS = NCORES * TROWS      # rows per (A or B) gather table
    HT = NTILES // 2            # tiles per half
    SGRP = SLOT // 128          # tiles per bounce-ship DMA (= tiles/slot)
    NGRP = HT // SGRP           # = GPC // 2
    assert NGRP * SGRP == HT
    sched = pre['sched']

    use_block = pre['use_block']
    is_start = pre['blk_start']
    is_stop = pre['blk_stop']

    # ---- I/O ----
    msgs0_d = nc.dram_tensor("msgs0", [NCH, 128, CBLK * D], f16,
                             kind="ExternalInput")
    xownT = nc.dram_tensor("xownT", [D, NCOL], f32, kind="ExternalInput")
    gidx_d = nc.dram_tensor("gidx", [NCH, 128, CPW], i16, kind="ExternalInput")
    S_d = nc.dram_tensor("S", [BLK, NBLK_TOT * 128], f8, kind="ExternalInput")
    dcol_d = nc.dram_tensor("dcol", [BLK, NBLK_TOT], f32, kind="ExternalInput")
    colidx_d = nc.dram_tensor("colidx", [128, 128], i8, kind="ExternalInput")
    diag_d = nc.dram_tensor("diag", [128, NCOL], bf16, kind="ExternalInput")
    mask_d = nc.dram_tensor("mask", [128, NCOL], f8, kind="ExternalInput")
    R2d_d = nc.dram_tensor("R2d", [2 * GPC, 2 * G], f32, kind="ExternalInput")
    gfT_d = nc.dram_tensor("gfT", [G_FEAT, G], f32, kind="ExternalInput")
    ident_d = nc.dram_tensor("ident", [128, 128], f32, kind="ExternalInput")
    Wl_d = [nc.dram_tensor(f"Wl{i}", [D, HID], f32, kind="ExternalInput")
            for i in range(3)]
    Wr_d = [nc.dram_tensor(f"Wr{i}", [D, HID], f32, kind="ExternalInput")
            for i in range(3)]
    gb_d = [nc.dram_tensor(f"gb{i}", [HID, 2], f32, kind="ExternalInput")
            for i in range(3)]
    W1_d = nc.dram_tensor("W1", [2 * HID + G_FEAT, HID], f32, kind="ExternalInput")
    W2_d = nc.dram_tensor("W2", [HID, HID // 2], f32, kind="ExternalInput")
    W3_d = nc.dram_tensor("W3", [HID // 2, 1], f32, kind="ExternalInput")
    bT_d = nc.dram_tensor("bT", [HID, 3], f32, kind="ExternalInput")

    out_d = nc.dram_tensor("out", [G], f32, kind="ExternalOutput")

    rg = [list(range(NCORES))]

    with tile.TileContext(nc) as tc:
        with (
            tc.tile_pool(name="sb", bufs=3) as sb,
            tc.tile_pool(name="big", bufs=2) as bigp,       # zT (f32 NCOL)
            tc.tile_pool(name="agg", bufs=1) as aggp,       # agg_sb bf16
            tc.tile_pool(name="big1", bufs=1) as big1,      # allp
            tc.tile_pool(name="msg", bufs=4) as msgp,       # msgs + S per chunk
            tc.tile_pool(name="idx", bufs=6) as idxp,
            tc.tile_pool(name="stg", bufs=2) as stgp,       # bounce staging
            tc.tile_pool(name="cst", bufs=1) as cst,
            tc.tile_pool(name="ps", bufs=2, space="PSUM") as ps,
            tc.tile_pool(name="pst", bufs=2, space="PSUM") as pst,
            tc.tile_pool(name="psa", bufs=2, space="PSUM") as psa,
            tc.tile_pool(name="psm", bufs=1, space="PSUM") as psm,
            tc.tile_pool(name="dram", bufs=1, space="DRAM") as dram,
        ):
            # ---- DRAM scratch ----
            hbA = [dram.tile([TROWS, D], f16, tag=f"hbA{i}", name=f"hbA{i}")
                   for i in range(2)]
            hbB = [dram.tile([TROWS, D], f16, tag=f"hbB{i}", name=f"hbB{i}")
                   for i in range(2)]
            tblA = [dram.tile([HROWS, D], f16, tag=f"tblA{i}",
                              name=f"tblA{i}", addr_space="Shared")
                    for i in range(2)]
            tblB = [dram.tile([HROWS, D], f16, tag=f"tblB{i}",
                              name=f"tblB{i}", addr_space="Shared")
                    for i in range(2)]
            stats_in = [dram.tile([D, 2], f32, tag=f"stats_in{i}",
                                  name=f"stats_in{i}") for i in range(3)]
            stats_out = [dram.tile([NCORES * D, 2], f32, tag=f"stats_out{i}",
                                   name=f"stats_out{i}", addr_space="Shared")
                         for i in range(3)]
            pool_in = dram.tile([D, 2 * G], f32, tag="pool_in")
            pool_out = dram.tile([NCORES * D, 2 * G], f32, tag="pool_out",
                                 addr_space="Shared")

            def load_const(src_ap, rows, cols, name, dt=f32):
                t = cst.tile([rows, cols], dt, tag=name)
                nc.sync.dma_start(out=t[:, :], in_=src_ap)
                return t

            ident_sb = load_const(ident_d[:, :], 128, 128, "ident")
            diag_sb = load_const(diag_d[:, :], 128, NCOL, "diag", bf16)
            mask_sb = load_const(mask_d[:, :], 128, NCOL, "mask", f8)
            dcol_sb = load_const(dcol_d[:, :], BLK, NBLK_TOT, "dcol")
            colidx_sb = load_const(colidx_d[:, :], 128, 128, "colidx", i8)
            xT_sb = bigp.tile([128, NCOL], f32, tag="zT")
            nc.sync.dma_start(out=xT_sb[:, :], in_=xownT[:, :])

            hT_prev = xT_sb

            # deferred AG_B trigger (emitted inside the NEXT layer's chunk
            # loop so A-chunk gathers overlap the B AllGather wire time)
            pending_agb = [None]

            def emit_agb():
                if pending_agb[0] is not None:
                    hb, tb = pending_agb[0]
                    nc.gpsimd.collective_compute(
                        "AllGather", mybir.AluOpType.bypass,
                        replica_groups=rg, ins=[hb.opt()], outs=[tb.opt()])
                    pending_agb[0] = None

            for li in range(3):
                Wl_sb = load_const(Wl_d[li][:, :], D, HID, f"Wl{li}")
                Wr_sb = load_const(Wr_d[li][:, :], D, HID, f"Wr{li}")
                gb_sb = load_const(gb_d[li][:, :], HID, 2, f"gb{li}")

                agg_sb = aggp.tile([128, NCOL], bf16, tag="agg")
                zT = bigp.tile([128, NCOL], f32, tag="zT")
                zsum = sb.tile([128, NTILES], f32, tag="zsum")
                zsq = sb.tile([128, NTILES], f32, tag="zsq")
                sq_scr = sb.tile([128, D], f32, tag="sqscr")

                def dense_tile(t, zT=zT, zsum=zsum, zsq=zsq, sq_scr=sq_scr,
                               agg_sb=agg_sb, Wl_sb=Wl_sb, Wr_sb=Wr_sb,
                               hT_prev=hT_prev):
                    aT_ps = pst.tile([128, D], f32, tag="tp")
                    # transpose fused with the 1/indeg column scale:
                    # out[f, j] = sum_k agg[k, f] * diag[k, j] = agg[j, f]/deg_j
                    nc.tensor.matmul(aT_ps[:, :],
                                     agg_sb[:, t * 128:(t + 1) * 128],
                                     diag_sb[:, t * 128:(t + 1) * 128],
                                     start=True, stop=True)
                    aT_sb = sb.tile([128, D], f32, tag="aTs")
                    nc.vector.tensor_copy(aT_sb[:, :], aT_ps[:, :])
                    z_ps = ps.tile([128, D], f32, tag="z")
                    nc.tensor.matmul(z_ps[:, :], Wl_sb[:, :], aT_sb[:, :],
                                     start=True, stop=False)
                    nc.tensor.matmul(z_ps[:, :], Wr_sb[:, :],
                                     hT_prev[:, t * 128:(t + 1) * 128],
                                     start=False, stop=True)
                    nc.scalar.activation(zT[:, t * 128:(t + 1) * 128],
                                         z_ps[:, :],
                                         mybir.ActivationFunctionType.Copy,
                                         accum_out=zsum[:, t:t + 1])
                    nc.scalar.activation(sq_scr[:, :],
                                         zT[:, t * 128:(t + 1) * 128],
                                         mybir.ActivationFunctionType.Square,
                                         accum_out=zsq[:, t:t + 1])

                # ---- gather + S-matmul aggregation, dense tiles interleaved
                # into the chunk loop as their aggregation completes (engine
                # queues execute in emission order - this is what overlaps
                # the dense phase with the gather pipeline) ----
                acc_ps = None
                gq = [0]
                tdone = 0
                for k in range(NCH):
                    h = pre['chunk_half'][k]
                    if li > 0 and h == 1:
                        emit_agb()   # half-B gathers need the B AllGather
                    msgs = msgp.tile([128, CBLK, D], f16, tag="msgs")
                    if li == 0:
                        nc.sync.dma_start(
                            out=msgs[:, :, :],
                            in_=msgs0_d[k, :, :].rearrange(
                                "p (j d) -> p j d", d=D))
                    else:
                        src_tab = (tblA if h == 0 else tblB)[(li - 1) % 2]
                        gi_t = idxp.tile([128, CPW], i16, tag="gi")
                        nc.sync.dma_start(out=gi_t[:], in_=gidx_d[k, :, :])
                        # runtime faults on dma_gather num_idxs > 1024:
                        # <=1024-index sub-gathers trimmed to per-run real
                        # extents; round-robin the 4 SWDGE queues
                        for off, n in pre['gather_pieces'][k]:
                            nc.gpsimd.dma_gather(
                                msgs[:, off // 128:off // 128
                                     + (n + 127) // 128, :],
                                src_tab[0:HROWS, :],
                                gi_t[:, off // 16:off // 16 + n // 16],
                                n, n, D, queue_num=gq[0] % NQ)
                            gq[0] += 1
                    # S blocks are one-hot.  Layer 0 streams them (its DMA
                    # has headroom); gather-bound layers 1/2 generate them
                    # on-chip (DVE is_equal of the column-iota against dcol)
                    S_t = msgp.tile([128, CBLK, D], f8, tag="Ssb")
                    if li == 0:
                        nc.scalar.dma_start(
                            out=S_t[:, :, :],
                            in_=S_d[:, k * CBLK * 128:(k + 1) * CBLK * 128]
                            .rearrange("p (j d) -> p j d", d=128))
                    else:
                        for j in range(CBLK):
                            b = k * CBLK + j
                            if not use_block[b]:
                                continue
                            nc.vector.scalar_tensor_tensor(
                                S_t[:, j, :], colidx_sb[:, :],
                                dcol_sb[:, b:b + 1], colidx_sb[:, :],
                                mybir.AluOpType.is_equal,
                                mybir.AluOpType.bypass)
                    for j in range(CBLK):
                        b = k * CBLK + j
                        if not use_block[b]:
                            continue
                        h_b, t_b = sched[b]
                        if is_start[b]:
                            acc_ps = psa.tile([128, D], f32, tag="accp")
                        nc.tensor.matmul(acc_ps[:, :], S_t[:, j, :],
                                         msgs[:, j, :],
                                         start=is_start[b], stop=is_stop[b])
                        if is_stop[b]:
                            if h_b == 0:
                                nc.scalar.copy(
                                    agg_sb[:, t_b * 128:(t_b + 1) * 128],
                                    acc_ps[:, :])
                            else:
                                nc.vector.tensor_add(
                                    agg_sb[:, t_b * 128:(t_b + 1) * 128],
                                    agg_sb[:, t_b * 128:(t_b + 1) * 128],
                                    acc_ps[:, :])
                    if li > 0 and k == TRIG_B_AFTER:
                        emit_agb()
                    while tdone < NTILES and pre['tile_ready'][tdone] <= k:
                        dense_tile(tdone)
                        tdone += 1
                emit_agb()
                while tdone < NTILES:
                    dense_tile(tdone)
                    tdone += 1

                stat_sb = sb.tile([128, 2], f32, tag="stat")
                nc.vector.tensor_reduce(stat_sb[:, 0:1], zsum[:, :],
                                        mybir.AxisListType.X,
                                        mybir.AluOpType.add)
                nc.vector.tensor_reduce(stat_sb[:, 1:2], zsq[:, :],
                                        mybir.AxisListType.X,
                                        mybir.AluOpType.add)
                nc.sync.dma_start(out=stats_in[li][:, :], in_=stat_sb[:, :])
                nc.gpsimd.collective_compute(
                    "AllGather", mybir.AluOpType.bypass, replica_groups=rg,
                    ins=[stats_in[li].opt()], outs=[stats_out[li].opt()])
                allst = sb.tile([128, NCORES, 2], f32, tag="allst")
                nc.sync.dma_start(
                    out=allst[:, :, :],
                    in_=stats_out[li][:, :].rearrange("(c p) j -> p c j",
                                                      c=NCORES))
                tot = sb.tile([128, 2], f32, tag="tot")
                nc.vector.tensor_add(tot[:, :], allst[:, 0, :], allst[:, 1, :])
                for c in range(2, NCORES):
                    nc.vector.tensor_add(tot[:, :], tot[:, :], allst[:, c, :])
                mu = sb.tile([128, 6], f32, tag="mu")
                nc.scalar.mul(mu[:, 0:1], tot[:, 0:1], 1.0 / N)
                nc.scalar.mul(mu[:, 1:2], tot[:, 1:2], 1.0 / N)
                nc.vector.tensor_mul(mu[:, 2:3], mu[:, 0:1], mu[:, 0:1])
                nc.vector.tensor_sub(mu[:, 3:4], mu[:, 1:2], mu[:, 2:3])
                nc.vector.tensor_scalar_add(mu[:, 3:4], mu[:, 3:4], EPS)
                nc.vector.reciprocal(mu[:, 4:5], mu[:, 3:4])
                nc.scalar.sqrt(mu[:, 4:5], mu[:, 4:5])
                nc.vector.tensor_mul(mu[:, 4:5], mu[:, 4:5], gb_sb[:, 0:1])
                nc.vector.tensor_mul(mu[:, 5:6], mu[:, 0:1], mu[:, 4:5])
                nc.vector.tensor_sub(mu[:, 5:6], gb_sb[:, 1:2], mu[:, 5:6])
                # h = relu(z*s + shift), pads zeroed via the fp8 column mask
                # (pads must stay 0 for BN stats + pooling), interleaved
                # per-tile with the transpose-ship / pooling consumers
                def apply_tile(t, zT=zT, mu=mu):
                    sl = slice(t * 128, (t + 1) * 128)
                    nc.scalar.activation(zT[:, sl], zT[:, sl],
                                         mybir.ActivationFunctionType.Relu,
                                         bias=mu[:, 5:6], scale=mu[:, 4:5])
                    nc.vector.tensor_mul(zT[:, sl], zT[:, sl], mask_sb[:, sl])
                hT_prev = zT

                if li < 2:
                    # ---- transpose back; ship f16 halves to split AG ----
                    for half, hb, tb in ((0, hbA[li % 2], tblA[li % 2]),
                                         (1, hbB[li % 2], tblB[li % 2])):
                        for grp in range(NGRP):
                            stg = stgp.tile([128, SGRP, 128], f16, tag="stg")
                            for j in range(SGRP):
                                t = half * HT + grp * SGRP + j
                                apply_tile(t)
                                hT_ps = pst.tile([128, D], f32, tag="tp")
                                nc.tensor.transpose(
                                    hT_ps[:, :],
                                    zT[:, t * 128:(t + 1) * 128],
                                    ident_sb[:, :])
                                nc.vector.tensor_copy(stg[:, j, :],
                                                      hT_ps[:, :])
                            r0 = grp * SGRP * 128
                            nc.sync.dma_start(
                                out=hb[r0:r0 + SGRP * 128, :].rearrange(
                                    "(j p) d -> p j d", p=128),
                                in_=stg[:, :, :])
                        if half == 0:
                            nc.gpsimd.collective_compute(
                                "AllGather", mybir.AluOpType.bypass,
                                replica_groups=rg,
                                ins=[hb.opt()], outs=[tb.opt()])
                        else:
                            pending_agb[0] = (hb, tb)
                else:
                    # ---- pooling: uniform per-slot column reduces ----
                    loc = sb.tile([128, 2 * GPC], f32, tag="loc")
                    for r in range(GPC):
                        for j in range(SGRP):
                            apply_tile(r * SGRP + j)
                        sl = slice(r * SLOT, (r + 1) * SLOT)
                        nc.vector.tensor_reduce(loc[:, r:r + 1], zT[:, sl],
                                                mybir.AxisListType.X,
                                                mybir.AluOpType.add)
                        nc.vector.tensor_reduce(loc[:, GPC + r:GPC + r + 1],
                                                zT[:, sl],
                                                mybir.AxisListType.X,
                                                mybir.AluOpType.max)
                    locT_ps = pst.tile([2 * GPC, 128], f32, tag="tp",
                                       name="locT")
                    nc.tensor.transpose(locT_ps[:, :], loc[:, :],
                                        ident_sb[:, :])
                    locT_sb = sb.tile([2 * GPC, 128], f32, tag="locTs")
                    nc.vector.tensor_copy(locT_sb[:, :], locT_ps[:, :])
                    R2d_sb = load_const(R2d_d[:, :], 2 * GPC, 2 * G, "R2d")
                    pool_ps = psm.tile([128, 2 * G], f32, tag="tail")
                    nc.tensor.matmul(pool_ps[:, :], locT_sb[:, :],
                                     R2d_sb[:, :], start=True, stop=True)

            # ---- pool partial exchange ----
            pool_sb = sb.tile([128, 2 * G], f32, tag="poolp")
            nc.vector.tensor_copy(pool_sb[:, :], pool_ps[:, :])
            nc.sync.dma_start(out=pool_in[:, :], in_=pool_sb[:, :])
            nc.gpsimd.collective_compute(
                "AllGather", mybir.AluOpType.bypass, replica_groups=rg,
                ins=[pool_in.opt()], outs=[pool_out.opt()])
            allp = big1.tile([128, NCORES, 2 * G], f32, tag="allp")
            nc.sync.dma_start(
                out=allp[:, :, :],
                in_=pool_out[:, :].rearrange("(c p) j -> p c j", c=NCORES))
            meanTot = sb.tile([128, G], f32, tag="meanTot")
            maxTot = sb.tile([128, G], f32, tag="maxTot")
            nc.vector.tensor_add(meanTot[:, :], allp[:, 0, 0:G],
                                 allp[:, 1, 0:G])
            nc.vector.tensor_max(maxTot[:, :], allp[:, 0, G:2 * G],
                                 allp[:, 1, G:2 * G])
            for c in range(2, NCORES):
                nc.vector.tensor_add(meanTot[:, :], meanTot[:, :],
                                     allp[:, c, 0:G])
                nc.vector.tensor_max(maxTot[:, :], maxTot[:, :],
                                     allp[:, c, G:2 * G])

            # ---- head (feature-major) ----
            W1a_sb = load_const(W1_d[0:HID, :], HID, HID, "W1a")
            W1b_sb = load_const(W1_d[HID:2 * HID, :], HID, HID, "W1b")
            W1c_sb = load_const(W1_d[2 * HID:2 * HID + G_FEAT, :], G_FEAT,
                                HID, "W1c")
            W2_sb = load_const(W2_d[:, :], HID, HID // 2, "W2")
            W3_sb = load_const(W3_d[:, :], HID // 2, 1, "W3")
            bT_sb = load_const(bT_d[:, :], HID, 3, "bT")
            gfT_sb = load_const(gfT_d[:, :], G_FEAT, G, "gfT")

            m1_ps = psm.tile([HID, G], f32, tag="tail")
            nc.tensor.matmul(m1_ps[:, :], W1a_sb[:, :], meanTot[:, :],
                             start=True, stop=False)
            nc.tensor.matmul(m1_ps[:, :], W1b_sb[:, :], maxTot[:, :],
                             start=False, stop=False)
            nc.tensor.matmul(m1_ps[:, :], W1c_sb[:, :],
                             gfT_sb[:, :], start=False, stop=True)
            m1_sb = sb.tile([HID, G], f32, tag="m1s")
            nc.scalar.activation(m1_sb[:, :], m1_ps[:, :],
                                 mybir.ActivationFunctionType.Relu,
                                 bias=bT_sb[:, 0:1])
            m2_ps = psm.tile([HID // 2, G], f32, tag="tail")
            nc.tensor.matmul(m2_ps[:, :], W2_sb[:, :], m1_sb[:, :],
                             start=True, stop=True)
            m2_sb = sb.tile([HID // 2, G], f32, tag="m2s")
            nc.scalar.activation(m2_sb[:, :], m2_ps[:, :],
                                 mybir.ActivationFunctionType.Relu,
                                 bias=bT_sb[0:HID // 2, 1:2])
            m3_ps = psm.tile([1, G], f32, tag="tail")
            nc.tensor.matmul(m3_ps[:, :], W3_sb[:, :], m2_sb[:, :],
                             start=True, stop=True)
            m3_sb = sb.tile([1, G], f32, tag="m3s")
            nc.scalar.copy(m3_sb[:, :], m3_ps[:, :])
            nc.vector.tensor_scalar_add(m3_sb[:, :], m3_sb[:, :],
                                        bT_sb[0:1, 2:3])
            nc.sync.dma_start(out=out_d[:].rearrange("(o g) -> o g", o=1),
                              in_=m3_sb[:, :])
    return nc


# ---------------- public entry ------------------------------------------------

def build_in_maps(x, edge_index, batch, g_feats, params, pre):
    x = np.asarray(x, dtype=np.float32)
    g_feats = np.asarray(g_feats, dtype=np.float32)
    batch = np.asarray(batch, dtype=np.int64)

    bT = np.zeros((HID, 3), np.float32)
    bT[:, 0] = np.asarray(params['b1'], np.float32)
    bT[:HID // 2, 1] = np.asarray(params['b2'], np.float32)
    bT[0, 2] = np.asarray(params['b3'], np.float32).reshape(-1)[0]

    common = {
        "ident": np.eye(128, dtype=np.float32),
        "colidx": np.broadcast_to(
            np.arange(128, dtype=np.int8), (128, 128)).copy(),
        "gfT": np.ascontiguousarray(g_feats.T),
        "W1": np.asarray(params['W1'], np.float32),
        "W2": np.asarray(params['W2'], np.float32),
        "W3": np.asarray(params['W3'], np.float32),
        "bT": bT,
    }
    for i in range(3):
        common[f"Wl{i}"] = np.asarray(params[f'Wl{i}'], np.float32)
        common[f"Wr{i}"] = np.asarray(params[f'Wr{i}'], np.float32)
        gb = np.zeros((HID, 2), np.float32)
        gb[:, 0] = np.asarray(params[f'gamma{i}'], np.float32)
        gb[:, 1] = np.asarray(params[f'beta{i}'], np.float32)
        common[f"gb{i}"] = gb

    x16 = x.astype(np.float16)
    NCH = pre['nchunks']
    SLOT, NCOL = pre['SLOT'], pre['NCOL']
    cnt_g, gstart = pre['cnt_g'], pre['gstart']
    in_maps = []
    for c in range(NCORES):
        xo = np.zeros((NCOL, D), np.float32)
        for r in range(GPC):
            g = c * GPC + r
            sz = int(cnt_g[g])
            xo[r * SLOT:r * SLOT + sz] = x[gstart[g]:gstart[g] + sz]
        # pre-gather layer-0 messages into the exact chunk SBUF layout:
        # slot s of chunk k -> partition s%128, free block s//128
        gi_abs = pre['giabs'][c]
        msgs0 = x16[gi_abs].reshape(NCH, CBLK, BLK, D).transpose(0, 2, 1, 3)
        msgs0 = np.ascontiguousarray(msgs0.reshape(NCH, 128, CBLK * D))
        m = dict(common)
        m.update({
            "xownT": np.ascontiguousarray(xo.T),
            "msgs0": msgs0,
            "gidx": pre['gidx'][c],
            "S": pre['S'][c],
            "dcol": pre['dcol'][c],
            "diag": pre['diag'][c],
            "mask": pre['mask'][c],
            "R2d": pre['R2d'][c],
        })
        in_maps.append(m)
    return in_maps


def build_nc(pre):
    import os
    import concourse.bacc as bacc
    nc = bacc.Bacc(None, target_bir_lowering=False, debug=False,
                   num_devices=NCORES, num_swdge_queues=4,
                   detect_race_conditions=os.environ.get(
                       "KERNEL_NO_RACE_CHECK") != "1")
    nc = _build(nc, pre)
    nc.compile()
    return nc


def kernel(x, edge_index, batch, g_feats,
           Wl0, bl0, Wr0, gamma0, beta0,
           Wl1, bl1, Wr1, gamma1, beta1,
           Wl2, bl2, Wr2, gamma2, beta2,
           W1, b1, W2, b2, W3, b3):
    # bl{i} cancels inside BatchNorm (constant pre-BN shift), so it is unused.
    from concourse.bass_utils import run_bass_kernel_spmd

    params = dict(Wl0=Wl0, Wr0=Wr0, gamma0=gamma0, beta0=beta0,
                  Wl1=Wl1, Wr1=Wr1, gamma1=gamma1, beta1=beta1,
                  Wl2=Wl2, Wr2=Wr2, gamma2=gamma2, beta2=beta2,
                  W1=W1, b1=b1, W2=W2, b2=b2, W3=W3, b3=b3)
    pre = _preprocess(x, edge_index, batch)
    nc = build_nc(pre)
    in_maps = build_in_maps(x, edge_index, batch, g_feats, params, pre)
    res = run_bass_kernel_spmd(nc, in_maps, list(range(NCORES)))
    return np.asarray(res.results[0]["out"], dtype=np.float32)


# revision 37
# speedup vs baseline: 1.1165x; 1.0061x over previous
"""Distributed Bass/Trainium2 kernel for nn_AreaGNN: 3x SAGEConv(mean) +
global BatchNorm + ReLU, per-graph mean/max pooling, 3-layer MLP head.
SPMD across 8 NeuronCores; takes FULL inputs, returns FULL output [G].

Node layout: batch is sorted, so graphs are contiguous; core c owns graphs
8c..8c+7, each in a fixed SLOT-column window of the feature-major zT (pads
zeroed by a per-core fp8 column mask after each ReLU).  Pooling is then just
uniform per-slot column reduces + one tiny route matmul - no transposes, no
slot gather.  Edges are owned by the dst graph's core, sorted by (src-half,
dst-tile), padded into 128-edge blocks with a global SPMD-uniform schedule.
Layer-0 messages are pre-gathered host-side and streamed; layer-1/2 messages
come from split (A/B) f16 AllGather node tables via hardware dma_gather over
4 SWDGE queues.  The A-half AllGather is issued mid-transpose-loop and the
B-half trigger is deferred into the next layer's chunk loop so gathers for
A-chunks overlap the B AllGather wire time.  Segment sums run on the
TensorEngine via per-block one-hot S matrices (fp8) accumulated in f32 PSUM;
the 1/indeg mean weighting is folded into the per-tile transpose as a
bf16 diagonal matmul.
"""
import numpy as np

N = 50000
E = 800000
D = 128
HID = 128
G = 64
G_FEAT = 32
EPS = 1e-5
NCORES = 8
GPC = G // NCORES           # 8 graphs per core
BLK = 128                   # edges per S block
CBLK = 32                   # blocks per gather chunk (4096 edges)
CH = BLK * CBLK
CPW = CH // 16
NQ = 4                      # SWDGE queues used for gathers
TRIG_B_AFTER = 6            # chunks of next layer emitted before AG_B trigger


# ---------------- host-side preprocessing -----------------------------------

def _wrap_idx(idx, ch):
    """[L] -> [L/ch, 128, ch/16] int16: element m of a chunk at (m%16, m//16),
    replicated across the eight 16-partition groups."""
    L = idx.shape[0]
    out = np.empty((L // ch, 128, ch // 16), dtype=np.int16)
    w = idx.reshape(L // ch, ch // 16, 16).transpose(0, 2, 1)
    for g in range(8):
        out[:, g * 16:(g + 1) * 16, :] = w
    return out


def _preprocess(x, edge_index, batch):
    src = np.asarray(edge_index[0], dtype=np.int64)
    dst = np.asarray(edge_index[1], dtype=np.int64)
    batch = np.asarray(batch, dtype=np.int64)

    # ---- node layout: graph g -> core g//GPC, slot g%GPC ----
    cnt_g = np.bincount(batch, minlength=G)
    assert cnt_g.min() >= 1, "empty graph unsupported"
    gstart = np.concatenate([[0], np.cumsum(cnt_g)])
    maxsz = int(cnt_g.max())
    SLOT = max(896, -(-(maxsz + 2) // 128) * 128)   # last col of every slot pad
    NCOL = GPC * SLOT
    NTILES = NCOL // 128
    NAH = NCOL // 2                                 # cols per table half
    TROWS = NAH                                     # rows per core per table
    assert NCORES * TROWS < 32768                   # int16 gather indices

    g_of = batch
    core_of_node = g_of // GPC
    newcol = (g_of % GPC) * SLOT + (np.arange(N) - gstart[g_of])

    indeg = np.bincount(dst, minlength=N)
    invdeg_all = (1.0 / np.maximum(indeg, 1.0)).astype(np.float32)

    dstc = core_of_node[dst]
    dcol = newcol[dst]
    tile_of = dcol // 128
    scol = newcol[src]
    score = core_of_node[src]
    half_of = (scol >= NAH).astype(np.int64)
    src_pos = score * TROWS + (scol - half_of * NAH)  # position in half table

    # per (core, half, tile) edge lists
    counts = np.zeros((NCORES, 2, NTILES), dtype=np.int64)
    buckets = {}
    for c in range(NCORES):
        mc = dstc == c
        for h in range(2):
            mh = mc & (half_of == h)
            for t in range(NTILES):
                m = mh & (tile_of == t)
                buckets[(c, h, t)] = (src_pos[m], dcol[m] % 128, src[m])
                counts[c, h, t] = int(m.sum())

    # global block schedule: NBLK[h, t] = max over cores; half A gets >= 1
    # block per tile so the pass-A psum->agg copy always initializes agg cols
    nblk = np.ceil(counts.max(axis=0) / BLK).astype(np.int64)  # [2, NTILES]
    nblk[0] = np.maximum(nblk[0], 1)
    extra = [0, 0]
    for h in range(2):
        tot = int(nblk[h].sum())
        extra[h] = (-tot) % CBLK
    sched = []   # list of (h, t) per block, in execution order
    for h in range(2):
        for t in range(NTILES):
            sched += [(h, t)] * int(nblk[h, t])
        sched += [(h, NTILES - 1)] * extra[h]
    nblk_tot = len(sched)
    nchunks = nblk_tot // CBLK
    assert nchunks * CBLK == nblk_tot
    chunk_half = [sched[k * CBLK][0] for k in range(nchunks)]
    for k in range(nchunks):
        assert all(sched[k * CBLK + j][0] == chunk_half[k] for j in range(CBLK))

    # Real (max-over-cores) block prefix of each (h,t) run: only these blocks
    # are matmul'd, and gather pieces cover exactly their units, so no stale
    # SBUF is ever read.  Pure-padding blocks are skipped entirely.
    run_start = {}
    b0 = 0
    for h in range(2):
        for t in range(NTILES):
            nb = int(nblk[h, t]) + (extra[h] if t == NTILES - 1 else 0)
            run_start[(h, t)] = (b0, nb)
            b0 += nb
    nbr = {}   # real blocks per run (>=1 for half A: initializes agg cols)
    for (h, t), (rb, nb) in run_start.items():
        n = -(-int(counts[:, h, t].max()) // BLK)
        if h == 0:
            n = max(n, 1)
        nbr[(h, t)] = min(n, nb)
    use_block = np.zeros(nblk_tot, dtype=bool)
    blk_start = np.zeros(nblk_tot, dtype=bool)
    blk_stop = np.zeros(nblk_tot, dtype=bool)
    for (h, t), (rb, nb) in run_start.items():
        n = nbr[(h, t)]
        if n > 0:
            use_block[rb:rb + n] = True
            blk_start[rb] = True
            blk_stop[rb + n - 1] = True
    # pieces trimmed to per-run real extents (128-aligned, <=1024 idx each)
    gather_pieces = []
    for k in range(nchunks):
        c0, c1 = k * CBLK * BLK, (k + 1) * CBLK * BLK
        pieces = []
        for (h, t), (rb, nb) in run_start.items():
            if h != chunk_half[k]:
                continue
            s0, s1 = rb * BLK, rb * BLK + nbr[(h, t)] * BLK
            a, b = max(s0, c0), min(s1, c1)
            for s in range(a, b, 1024):
                pieces.append((s - c0, min(1024, b - s)))
        gather_pieces.append(sorted(pieces))
    # chunk after which tile t's aggregation is complete
    last_blk = {}
    for (h, t), (rb, nb) in run_start.items():
        n = nbr[(h, t)]
        if n > 0:
            last_blk[t] = max(last_blk.get(t, -1), rb + n - 1)
    tile_ready = [last_blk[t] // CBLK for t in range(NTILES)]

    # per-core gather idx + per-slot dst columns following the schedule.
    # S blocks are pure one-hot rows, generated ON-CHIP per block via a DVE
    # is_equal against dcol (slot -> dst column, -1 for pad slots); the
    # 1/indeg mean weight is applied later by a diagonal-matmul fused into
    # the per-tile transpose.
    import ml_dtypes
    gidx_cores, dcol_cores, giabs_cores, diag_cores = [], [], [], []
    mask_cores, R2d_cores, S_cores = [], [], []
    inv_cnt = (1.0 / cnt_g.astype(np.float64)).astype(np.float32)
    for c in range(NCORES):
        gi = np.zeros(nblk_tot * BLK, dtype=np.int64)
        ga = np.zeros(nblk_tot * BLK, dtype=np.int64)   # abs src (msgs0)
        dc = np.full(nblk_tot * BLK, -1.0, dtype=np.float32)
        b0 = 0
        for h in range(2):
            for t in range(NTILES):
                nb = int(nblk[h, t]) + (extra[h] if t == NTILES - 1 else 0)
                g, d, a = buckets[(c, h, t)]
                n = len(g)
                gi[b0 * BLK: b0 * BLK + n] = g
                ga[b0 * BLK: b0 * BLK + n] = a
                dc[b0 * BLK: b0 * BLK + n] = d
                b0 += nb
        assert b0 == nblk_tot
        giabs_cores.append(ga)
        gidx_cores.append(_wrap_idx(gi.astype(np.int16), CH))
        # dcol layout [slot-in-block (partition), block]
        dcol_cores.append(np.ascontiguousarray(
            dc.reshape(nblk_tot, BLK).T))
        # dense one-hot S, streamed only by layer 0 (DMA has headroom there;
        # gather-bound layers 1/2 generate S on-chip instead)
        S = np.zeros((nblk_tot * BLK, 128), dtype=np.float32)
        rows = np.arange(nblk_tot * BLK)
        real = dc >= 0
        S[rows[real], dc[real].astype(np.int64)] = 1.0
        S_cores.append(np.ascontiguousarray(
            S.reshape(nblk_tot, BLK, 128).transpose(1, 0, 2)
            .reshape(BLK, nblk_tot * 128)).astype(ml_dtypes.float8_e4m3))

        iv = np.ones(NCOL, np.float32)
        msk = np.zeros(NCOL, np.float32)
        for r in range(GPC):
            g = c * GPC + r
            sz = int(cnt_g[g])
            iv[r * SLOT:r * SLOT + sz] = \
                invdeg_all[gstart[g]:gstart[g] + sz]
            msk[r * SLOT:r * SLOT + sz] = 1.0
        dg = np.zeros((128, NCOL), dtype=np.float32)
        for t in range(NTILES):
            dg[np.arange(128), t * 128 + np.arange(128)] = \
                iv[t * 128:(t + 1) * 128]
        diag_cores.append(dg.astype(ml_dtypes.bfloat16))
        mask_cores.append(
            np.broadcast_to(msk.astype(ml_dtypes.float8_e4m3),
                            (128, NCOL)).copy())

        R2d = np.zeros((2 * GPC, 2 * G), dtype=np.float32)
        for r in range(GPC):
            g = c * GPC + r
            R2d[r, g] = inv_cnt[g]
            R2d[GPC + r, G + g] = 1.0
        R2d_cores.append(R2d)

    return dict(SLOT=SLOT, NCOL=NCOL, NTILES=NTILES, NAH=NAH, TROWS=TROWS,
                cnt_g=cnt_g, gstart=gstart,
                nblk=nblk, extra=extra, sched=sched, nblk_tot=nblk_tot,
                nchunks=nchunks, chunk_half=chunk_half,
                use_block=use_block, blk_start=blk_start, blk_stop=blk_stop,
                gather_pieces=gather_pieces, tile_ready=tile_ready,
                gidx=gidx_cores, giabs=giabs_cores, dcol=dcol_cores,
                S=S_cores,
                diag=diag_cores, mask=mask_cores, R2d=R2d_cores)


# ---------------- device kernel builder --------------------------------------

def _build(nc, pre):
    import concourse.mybir as mybir
    import concourse.tile as tile

    f32 = mybir.dt.float32
    f16 = mybir.dt.float16
    bf16 = mybir.dt.bfloat16
    f8 = mybir.dt.float8e4
    i16 = mybir.dt.int16
    i8 = mybir.dt.int8
    NCH = pre['nchunks']
    NBLK_TOT = pre['nblk_tot']
    NCOL, NTILES, NAH, SLOT = \
        pre['NCOL'], pre['NTILES'], pre['NAH'], pre['SLOT']
    TROWS = pre['TROWS']
    HROWS = NCORES * TROWS      # rows per (A or B) gather table
    HT = NTILES // 2            # tiles per half
    SGRP = SLOT // 128          # tiles per bounce-ship DMA (= tiles/slot)
    NGRP = HT // SGRP           # = GPC // 2
    assert NGRP * SGRP == HT
    sched = pre['sched']

    use_block = pre['use_block']
    is_start = pre['blk_start']
    is_stop = pre['blk_stop']

    # ---- I/O ----
    msgs0_d = nc.dram_tensor("msgs0", [NCH, 128, CBLK * D], f16,
                             kind="ExternalInput")
    xownT = nc.dram_tensor("xownT", [D, NCOL], f32, kind="ExternalInput")
    gidx_d = nc.dram_tensor("gidx", [NCH, 128, CPW], i16, kind="ExternalInput")
    S_d = nc.dram_tensor("S", [BLK, NBLK_TOT * 128], f8, kind="ExternalInput")
    dcol_d = nc.dram_tensor("dcol", [BLK, NBLK_TOT], f32, kind="ExternalInput")
    colidx_d = nc.dram_tensor("colidx", [128, 128], i8, kind="ExternalInput")
    diag_d = nc.dram_tensor("diag", [128, NCOL], bf16, kind="ExternalInput")
    mask_d = nc.dram_tensor("mask", [128, NCOL], f8, kind="ExternalInput")
    R2d_d = nc.dram_tensor("R2d", [2 * GPC, 2 * G], f32, kind="ExternalInput")
    gfT_d = nc.dram_tensor("gfT", [G_FEAT, G], f32, kind="ExternalInput")
    ident_d = nc.dram_tensor("ident", [128, 128], f32, kind="ExternalInput")
    Wl_d = [nc.dram_tensor(f"Wl{i}", [D, HID], f32, kind="ExternalInput")
            for i in range(3)]
    Wr_d = [nc.dram_tensor(f"Wr{i}", [D, HID], f32, kind="ExternalInput")
            for i in range(3)]
    gb_d = [nc.dram_tensor(f"gb{i}", [HID, 2], f32, kind="ExternalInput")
            for i in range(3)]
    W1_d = nc.dram_tensor("W1", [2 * HID + G_FEAT, HID], f32, kind="ExternalInput")
    W2_d = nc.dram_tensor("W2", [HID, HID // 2], f32, kind="ExternalInput")
    W3_d = nc.dram_tensor("W3", [HID // 2, 1], f32, kind="ExternalInput")
    bT_d = nc.dram_tensor("bT", [HID, 3], f32, kind="ExternalInput")

    out_d = nc.dram_tensor("out", [G], f32, kind="ExternalOutput")

    rg = [list(range(NCORES))]

    with tile.TileContext(nc) as tc:
        with (
            tc.tile_pool(name="sb", bufs=3) as sb,
            tc.tile_pool(name="big", bufs=2) as bigp,       # zT (f32 NCOL)
            tc.tile_pool(name="agg", bufs=1) as aggp,       # agg_sb bf16
            tc.tile_pool(name="big1", bufs=1) as big1,      # allp
            tc.tile_pool(name="msg", bufs=3) as msgp,       # msgs + S per chunk
            tc.tile_pool(name="idx", bufs=6) as idxp,
            tc.tile_pool(name="stg", bufs=2) as stgp,       # bounce staging
            tc.tile_pool(name="cst", bufs=1) as cst,
            tc.tile_pool(name="ps", bufs=2, space="PSUM") as ps,
            tc.tile_pool(name="pst", bufs=2, space="PSUM") as pst,
            tc.tile_pool(name="psa", bufs=2, space="PSUM") as psa,
            tc.tile_pool(name="psm", bufs=1, space="PSUM") as psm,
            tc.tile_pool(name="dram", bufs=1, space="DRAM") as dram,
        ):
            # ---- DRAM scratch ----
            hbA = [dram.tile([TROWS, D], f16, tag=f"hbA{i}", name=f"hbA{i}")
                   for i in range(2)]
            hbB = [dram.tile([TROWS, D], f16, tag=f"hbB{i}", name=f"hbB{i}")
                   for i in range(2)]
            tblA = [dram.tile([HROWS, D], f16, tag=f"tblA{i}",
                              name=f"tblA{i}", addr_space="Shared")
                    for i in range(2)]
            tblB = [dram.tile([HROWS, D], f16, tag=f"tblB{i}",
                              name=f"tblB{i}", addr_space="Shared")
                    for i in range(2)]
            stats_in = [dram.tile([D, 2], f32, tag=f"stats_in{i}",
                                  name=f"stats_in{i}") for i in range(3)]
            stats_out = [dram.tile([NCORES * D, 2], f32, tag=f"stats_out{i}",
                                   name=f"stats_out{i}", addr_space="Shared")
                         for i in range(3)]
            pool_in = dram.tile([D, 2 * G], f32, tag="pool_in")
            pool_out = dram.tile([NCORES * D, 2 * G], f32, tag="pool_out",
                                 addr_space="Shared")

            def load_const(src_ap, rows, cols, name, dt=f32):
                t = cst.tile([rows, cols], dt, tag=name)
                nc.sync.dma_start(out=t[:, :], in_=src_ap)
                return t

            ident_sb = load_const(ident_d[:, :], 128, 128, "ident")
            diag_sb = load_const(diag_d[:, :], 128, NCOL, "diag", bf16)
            mask_sb = load_const(mask_d[:, :], 128, NCOL, "mask", f8)
            dcol_sb = load_const(dcol_d[:, :], BLK, NBLK_TOT, "dcol")
            colidx_sb = load_const(colidx_d[:, :], 128, 128, "colidx", i8)
            xT_sb = bigp.tile([128, NCOL], f32, tag="zT")
            nc.sync.dma_start(out=xT_sb[:, :], in_=xownT[:, :])

            hT_prev = xT_sb

            # deferred AG_B trigger (emitted inside the NEXT layer's chunk
            # loop so A-chunk gathers overlap the B AllGather wire time)
            pending_agb = [None]

            def emit_agb():
                if pending_agb[0] is not None:
                    hb, tb = pending_agb[0]
                    nc.gpsimd.collective_compute(
                        "AllGather", mybir.AluOpType.bypass,
                        replica_groups=rg, ins=[hb.opt()], outs=[tb.opt()])
                    pending_agb[0] = None

            for li in range(3):
                Wl_sb = load_const(Wl_d[li][:, :], D, HID, f"Wl{li}")
                Wr_sb = load_const(Wr_d[li][:, :], D, HID, f"Wr{li}")
                gb_sb = load_const(gb_d[li][:, :], HID, 2, f"gb{li}")

                agg_sb = aggp.tile([128, NCOL], bf16, tag="agg")
                zT = bigp.tile([128, NCOL], f32, tag="zT")
                zsum = sb.tile([128, NTILES], f32, tag="zsum")
                zsq = sb.tile([128, NTILES], f32, tag="zsq")
                sq_scr = sb.tile([128, D], f32, tag="sqscr")

                def dense_tile(t, zT=zT, zsum=zsum, zsq=zsq, sq_scr=sq_scr,
                               agg_sb=agg_sb, Wl_sb=Wl_sb, Wr_sb=Wr_sb,
                               hT_prev=hT_prev):
                    aT_ps = pst.tile([128, D], f32, tag="tp")
                    # transpose fused with the 1/indeg column scale:
                    # out[f, j] = sum_k agg[k, f] * diag[k, j] = agg[j, f]/deg_j
                    nc.tensor.matmul(aT_ps[:, :],
                                     agg_sb[:, t * 128:(t + 1) * 128],
                                     diag_sb[:, t * 128:(t + 1) * 128],
                                     start=True, stop=True)
                    aT_sb = sb.tile([128, D], f32, tag="aTs")
                    nc.vector.tensor_copy(aT_sb[:, :], aT_ps[:, :])
                    z_ps = ps.tile([128, D], f32, tag="z")
                    nc.tensor.matmul(z_ps[:, :], Wl_sb[:, :], aT_sb[:, :],
                                     start=True, stop=False)
                    nc.tensor.matmul(z_ps[:, :], Wr_sb[:, :],
                                     hT_prev[:, t * 128:(t + 1) * 128],
                                     start=False, stop=True)
                    nc.scalar.activation(zT[:, t * 128:(t + 1) * 128],
                                         z_ps[:, :],
                                         mybir.ActivationFunctionType.Copy,
                                         accum_out=zsum[:, t:t + 1])
                    nc.scalar.activation(sq_scr[:, :],
                                         zT[:, t * 128:(t + 1) * 128],
                                         mybir.ActivationFunctionType.Square,
                                         accum_out=zsq[:, t:t + 1])

                # ---- gather + S-matmul aggregation, dense tiles interleaved
                # into the chunk loop as their aggregation completes (engine
                # queues execute in emission order - this is what overlaps
                # the dense phase with the gather pipeline) ----
                acc_ps = None
                gq = [0]
                tdone = 0
                for k in range(NCH):
                    h = pre['chunk_half'][k]
                    if li > 0 and h == 1:
                        emit_agb()   # half-B gathers need the B AllGather
                    msgs = msgp.tile([128, CBLK, D], f16, tag="msgs")
                    if li == 0:
                        nc.sync.dma_start(
                            out=msgs[:, :, :],
                            in_=msgs0_d[k, :, :].rearrange(
                                "p (j d) -> p j d", d=D))
                    else:
                        src_tab = (tblA if h == 0 else tblB)[(li - 1) % 2]
                        gi_t = idxp.tile([128, CPW], i16, tag="gi")
                        nc.sync.dma_start(out=gi_t[:], in_=gidx_d[k, :, :])
                        # runtime faults on dma_gather num_idxs > 1024:
                        # <=1024-index sub-gathers trimmed to per-run real
                        # extents; round-robin the 4 SWDGE queues
                        for off, n in pre['gather_pieces'][k]:
                            nc.gpsimd.dma_gather(
                                msgs[:, off // 128:off // 128
                                     + (n + 127) // 128, :],
                                src_tab[0:HROWS, :],
                                gi_t[:, off // 16:off // 16 + n // 16],
                                n, n, D, queue_num=gq[0] % NQ)
                            gq[0] += 1
                    # S blocks are one-hot.  Layer 0 streams them (its DMA
                    # has headroom); gather-bound layers 1/2 generate them
                    # on-chip (DVE is_equal of the column-iota against dcol)
                    S_t = msgp.tile([128, CBLK, D], f8, tag="Ssb")
                    if li == 0:
                        nc.scalar.dma_start(
                            out=S_t[:, :, :],
                            in_=S_d[:, k * CBLK * 128:(k + 1) * CBLK * 128]
                            .rearrange("p (j d) -> p j d", d=128))
                    else:
                        for j in range(CBLK):
                            b = k * CBLK + j
                            if not use_block[b]:
                                continue
                            nc.vector.scalar_tensor_tensor(
                                S_t[:, j, :], colidx_sb[:, :],
                                dcol_sb[:, b:b + 1], colidx_sb[:, :],
                                mybir.AluOpType.is_equal,
                                mybir.AluOpType.bypass)
                    for j in range(CBLK):
                        b = k * CBLK + j
                        if not use_block[b]:
                            continue
                        h_b, t_b = sched[b]
                        if is_start[b]:
                            acc_ps = psa.tile([128, D], f32, tag="accp")
                        nc.tensor.matmul(acc_ps[:, :], S_t[:, j, :],
                                         msgs[:, j, :],
                                         start=is_start[b], stop=is_stop[b])
                        if is_stop[b]:
                            if h_b == 0:
                                nc.scalar.copy(
                                    agg_sb[:, t_b * 128:(t_b + 1) * 128],
                                    acc_ps[:, :])
                            else:
                                nc.vector.tensor_add(
                                    agg_sb[:, t_b * 128:(t_b + 1) * 128],
                                    agg_sb[:, t_b * 128:(t_b + 1) * 128],
                                    acc_ps[:, :])
                    if li > 0 and k == TRIG_B_AFTER:
                        emit_agb()
                    while tdone < NTILES and pre['tile_ready'][tdone] <= k:
                        dense_tile(tdone)
                        tdone += 1
                emit_agb()
                while tdone < NTILES:
                    dense_tile(tdone)
                    tdone += 1

                stat_sb = sb.tile([128, 2], f32, tag="stat")
                nc.vector.tensor_reduce(stat_sb[:, 0:1], zsum[:, :],
                                        mybir.AxisListType.X,
                                        mybir.AluOpType.add)
                nc.vector.tensor_reduce(stat_sb[:, 1:2], zsq[:, :],
                                        mybir.AxisListType.X,
                                        mybir.AluOpType.add)
                nc.sync.dma_start(out=stats_in[li][:, :], in_=stat_sb[:, :])
                nc.gpsimd.collective_compute(
                    "AllGather", mybir.AluOpType.bypass, replica_groups=rg,
                    ins=[stats_in[li].opt()], outs=[stats_out[li].opt()])
                allst = sb.tile([128, NCORES, 2], f32, tag="allst")
                nc.sync.dma_start(
                    out=allst[:, :, :],
                    in_=stats_out[li][:, :].rearrange("(c p) j -> p c j",
                                                      c=NCORES))
                tot = sb.tile([128, 2], f32, tag="tot")
                nc.vector.tensor_add(tot[:, :], allst[:, 0, :], allst[:, 1, :])
                for c in range(2, NCORES):
                    nc.vector.tensor_add(tot[:, :], tot[:, :], allst[:, c, :])
                mu = sb.tile([128, 6], f32, tag="mu")
                nc.scalar.mul(mu[:, 0:1], tot[:, 0:1], 1.0 / N)
                nc.scalar.mul(mu[:, 1:2], tot[:, 1:2], 1.0 / N)
                nc.vector.tensor_mul(mu[:, 2:3], mu[:, 0:1], mu[:, 0:1])
                nc.vector.tensor_sub(mu[:, 3:4], mu[:, 1:2], mu[:, 2:3])
                nc.vector.tensor_scalar_add(mu[:, 3:4], mu[:, 3:4], EPS)
                nc.vector.reciprocal(mu[:, 4:5], mu[:, 3:4])
                nc.scalar.sqrt(mu[:, 4:5], mu[:, 4:5])
                nc.vector.tensor_mul(mu[:, 4:5], mu[:, 4:5], gb_sb[:, 0:1])
                nc.vector.tensor_mul(mu[:, 5:6], mu[:, 0:1], mu[:, 4:5])
                nc.vector.tensor_sub(mu[:, 5:6], gb_sb[:, 1:2], mu[:, 5:6])
                # h = relu(z*s + shift), pads zeroed via the fp8 column mask
                # (pads must stay 0 for BN stats + pooling), interleaved
                # per-tile with the transpose-ship / pooling consumers
                def apply_tile(t, zT=zT, mu=mu):
                    sl = slice(t * 128, (t + 1) * 128)
                    nc.scalar.activation(zT[:, sl], zT[:, sl],
                                         mybir.ActivationFunctionType.Relu,
                                         bias=mu[:, 5:6], scale=mu[:, 4:5])
                    nc.vector.tensor_mul(zT[:, sl], zT[:, sl], mask_sb[:, sl])
                hT_prev = zT

                if li < 2:
                    # ---- transpose back; ship f16 halves to split AG ----
                    for half, hb, tb in ((0, hbA[li % 2], tblA[li % 2]),
                                         (1, hbB[li % 2], tblB[li % 2])):
                        for grp in range(NGRP):
                            stg = stgp.tile([128, SGRP, 128], f16, tag="stg")
                            for j in range(SGRP):
                                t = half * HT + grp * SGRP + j
                                apply_tile(t)
                                hT_ps = pst.tile([128, D], f32, tag="tp")
                                nc.tensor.transpose(
                                    hT_ps[:, :],
                                    zT[:, t * 128:(t + 1) * 128],
                                    ident_sb[:, :])
                                nc.vector.tensor_copy(stg[:, j, :],
                                                      hT_ps[:, :])
                            r0 = grp * SGRP * 128
                            nc.sync.dma_start(
                                out=hb[r0:r0 + SGRP * 128, :].rearrange(
                                    "(j p) d -> p j d", p=128),
                                in_=stg[:, :, :])
                        if half == 0:
                            nc.gpsimd.collective_compute(
                                "AllGather", mybir.AluOpType.bypass,
                                replica_groups=rg,
                                ins=[hb.opt()], outs=[tb.opt()])
                        else:
                            pending_agb[0] = (hb, tb)
                else:
                    # ---- pooling: uniform per-slot column reduces ----
                    loc = sb.tile([128, 2 * GPC], f32, tag="loc")
                    for r in range(GPC):
                        for j in range(SGRP):
                            apply_tile(r * SGRP + j)
                        sl = slice(r * SLOT, (r + 1) * SLOT)
                        nc.vector.tensor_reduce(loc[:, r:r + 1], zT[:, sl],
                                                mybir.AxisListType.X,
                                                mybir.AluOpType.add)
                        nc.vector.tensor_reduce(loc[:, GPC + r:GPC + r + 1],
                                                zT[:, sl],
                                                mybir.AxisListType.X,
                                                mybir.AluOpType.max)
                    locT_ps = pst.tile([2 * GPC, 128], f32, tag="tp",
                                       name="locT")
                    nc.tensor.transpose(locT_ps[:, :], loc[:, :],
                                        ident_sb[:, :])
                    locT_sb = sb.tile([2 * GPC, 128], f32, tag="locTs")
                    nc.vector.tensor_copy(locT_sb[:, :], locT_ps[:, :])
                    R2d_sb = load_const(R2d_d[:, :], 2 * GPC, 2 * G, "R2d")
                    pool_ps = psm.tile([128, 2 * G], f32, tag="tail")
                    nc.tensor.matmul(pool_ps[:, :], locT_sb[:, :],
                                     R2d_sb[:, :], start=True, stop=True)

            # ---- pool partial exchange ----
            pool_sb = sb.tile([128, 2 * G], f32, tag="poolp")
            nc.vector.tensor_copy(pool_sb[:, :], pool_ps[:, :])
            nc.sync.dma_start(out=pool_in[:, :], in_=pool_sb[:, :])
            nc.gpsimd.collective_compute(
                "AllGather", mybir.AluOpType.bypass, replica_groups=rg,
                ins=[pool_in.opt()], outs=[pool_out.opt()])
            allp = big1.tile([128, NCORES, 2 * G], f32, tag="allp")
            nc.sync.dma_start(
                out=allp[:, :, :],
                in_=pool_out[:, :].rearrange("(c p) j -> p c j", c=NCORES))
            meanTot = sb.tile([128, G], f32, tag="meanTot")
            maxTot = sb.tile([128, G], f32, tag="maxTot")
            nc.vector.tensor_add(meanTot[:, :], allp[:, 0, 0:G],
                                 allp[:, 1, 0:G])
            nc.vector.tensor_max(maxTot[:, :], allp[:, 0, G:2 * G],
                                 allp[:, 1, G:2 * G])
            for c in range(2, NCORES):
                nc.vector.tensor_add(meanTot[:, :], meanTot[:, :],
                                     allp[:, c, 0:G])
                nc.vector.tensor_max(maxTot[:, :], maxTot[:, :],
                                     allp[:, c, G:2 * G])

            # ---- head (feature-major) ----
            W1a_sb = load_const(W1_d[0:HID, :], HID, HID, "W1a")
            W1b_sb = load_const(W1_d[HID:2 * HID, :], HID, HID, "W1b")
            W1c_sb = load_const(W1_d[2 * HID:2 * HID + G_FEAT, :], G_FEAT,
                                HID, "W1c")
            W2_sb = load_const(W2_d[:, :], HID, HID // 2, "W2")
            W3_sb = load_const(W3_d[:, :], HID // 2, 1, "W3")
            bT_sb = load_const(bT_d[:, :], HID, 3, "bT")
            gfT_sb = load_const(gfT_d[:, :], G_FEAT, G, "gfT")

            m1_ps = psm.tile([HID, G], f32, tag="tail")
            nc.tensor.matmul(m1_ps[:, :], W1a_sb[:, :], meanTot[:, :],
                             start=True, stop=False)
            nc.tensor.matmul(m1_ps[:, :], W1b_sb[:, :], maxTot[:, :],
                             start=False, stop=False)
            nc.tensor.matmul(m1_ps[:, :], W1c_sb[:, :],
                             gfT_sb[:, :], start=False, stop=True)
            m1_sb = sb.tile([HID, G], f32, tag="m1s")
            nc.scalar.activation(m1_sb[:, :], m1_ps[:, :],
                                 mybir.ActivationFunctionType.Relu,
                                 bias=bT_sb[:, 0:1])
            m2_ps = psm.tile([HID // 2, G], f32, tag="tail")
            nc.tensor.matmul(m2_ps[:, :], W2_sb[:, :], m1_sb[:, :],
                             start=True, stop=True)
            m2_sb = sb.tile([HID // 2, G], f32, tag="m2s")
            nc.scalar.activation(m2_sb[:, :], m2_ps[:, :],
                                 mybir.ActivationFunctionType.Relu,
                                 bias=bT_sb[0:HID // 2, 1:2])
            m3_ps = psm.tile([1, G], f32, tag="tail")
            nc.tensor.matmul(m3_ps[:, :], W3_sb[:, :], m2_sb[:, :],
                             start=True, stop=True)
            m3_sb = sb.tile([1, G], f32, tag="m3s")
            nc.scalar.copy(m3_sb[:, :], m3_ps[:, :])
            nc.vector.tensor_scalar_add(m3_sb[:, :], m3_sb[:, :],
                                        bT_sb[0:1, 2:3])
            nc.sync.dma_start(out=out_d[:].rearrange("(o g) -> o g", o=1),
                              in_=m3_sb[:, :])
    return nc


# ---------------- public entry ------------------------------------------------

def build_in_maps(x, edge_index, batch, g_feats, params, pre):
    x = np.asarray(x, dtype=np.float32)
    g_feats = np.asarray(g_feats, dtype=np.float32)
    batch = np.asarray(batch, dtype=np.int64)

    bT = np.zeros((HID, 3), np.float32)
    bT[:, 0] = np.asarray(params['b1'], np.float32)
    bT[:HID // 2, 1] = np.asarray(params['b2'], np.float32)
    bT[0, 2] = np.asarray(params['b3'], np.float32).reshape(-1)[0]

    common = {
        "ident": np.eye(128, dtype=np.float32),
        "colidx": np.broadcast_to(
            np.arange(128, dtype=np.int8), (128, 128)).copy(),
        "gfT": np.ascontiguousarray(g_feats.T),
        "W1": np.asarray(params['W1'], np.float32),
        "W2": np.asarray(params['W2'], np.float32),
        "W3": np.asarray(params['W3'], np.float32),
        "bT": bT,
    }
    for i in range(3):
        common[f"Wl{i}"] = np.asarray(params[f'Wl{i}'], np.float32)
        common[f"Wr{i}"] = np.asarray(params[f'Wr{i}'], np.float32)
        gb = np.zeros((HID, 2), np.float32)
        gb[:, 0] = np.asarray(params[f'gamma{i}'], np.float32)
        gb[:, 1] = np.asarray(params[f'beta{i}'], np.float32)
        common[f"gb{i}"] = gb

    x16 = x.astype(np.float16)
    NCH = pre['nchunks']
    SLOT, NCOL = pre['SLOT'], pre['NCOL']
    cnt_g, gstart = pre['cnt_g'], pre['gstart']
    in_maps = []
    for c in range(NCORES):
        xo = np.zeros((NCOL, D), np.float32)
        for r in range(GPC):
            g = c * GPC + r
            sz = int(cnt_g[g])
            xo[r * SLOT:r * SLOT + sz] = x[gstart[g]:gstart[g] + sz]
        # pre-gather layer-0 messages into the exact chunk SBUF layout:
        # slot s of chunk k -> partition s%128, free block s//128
        gi_abs = pre['giabs'][c]
        msgs0 = x16[gi_abs].reshape(NCH, CBLK, BLK, D).transpose(0, 2, 1, 3)
        msgs0 = np.ascontiguousarray(msgs0.reshape(NCH, 128, CBLK * D))
        m = dict(common)
        m.update({
            "xownT": np.ascontiguousarray(xo.T),
            "msgs0": msgs0,
            "gidx": pre['gidx'][c],
            "S": pre['S'][c],
            "dcol": pre['dcol'][c],
            "diag": pre['diag'][c],
            "mask": pre['mask'][c],
            "R2d": pre['R2d'][c],
        })
        in_maps.append(m)
    return in_maps


def build_nc(pre):
    import os
    import concourse.bacc as bacc
    nc = bacc.Bacc(None, target_bir_lowering=False, debug=False,
                   num_devices=NCORES, num_swdge_queues=4,
                   dynamic_dma_scratch_size=32768,
                   detect_race_conditions=os.environ.get(
                       "KERNEL_NO_RACE_CHECK") != "1")
    nc = _build(nc, pre)
    nc.compile()
    return nc


def kernel(x, edge_index, batch, g_feats,
           Wl0, bl0, Wr0, gamma0, beta0,
           Wl1, bl1, Wr1, gamma1, beta1,
           Wl2, bl2, Wr2, gamma2, beta2,
           W1, b1, W2, b2, W3, b3):
    # bl{i} cancels inside BatchNorm (constant pre-BN shift), so it is unused.
    from concourse.bass_utils import run_bass_kernel_spmd

    params = dict(Wl0=Wl0, Wr0=Wr0, gamma0=gamma0, beta0=beta0,
                  Wl1=Wl1, Wr1=Wr1, gamma1=gamma1, beta1=beta1,
                  Wl2=Wl2, Wr2=Wr2, gamma2=gamma2, beta2=beta2,
                  W1=W1, b1=b1, W2=W2, b2=b2, W3=W3, b3=b3)
    pre = _preprocess(x, edge_index, batch)
    nc = build_nc(pre)
    in_maps = build_in_maps(x, edge_index, batch, g_feats, params, pre)
    res = run_bass_kernel_spmd(nc, in_maps, list(range(NCORES)))
    return np.asarray(res.results[0]["out"], dtype=np.float32)
